# revision 17
# baseline (speedup 1.0000x reference)
"""AttentiveChildSumTreeLSTMCell on 8 Trainium2 NeuronCores.

Cross-core communication via peer-to-peer SWDGE remote DMA (SBUF->SBUF)
instead of ncfw collectives: each core XOR-broadcasts its contribution into
per-source slot buffers on every peer (slot j of core r receives from the
core with tpb(r)^j), so a sum over slots implements AllReduce and the slot
buffer itself is an AllGather (ordered on the tpb-0 core).  Two exchanges:

  X1 [128, 528] bf16: merge-projection partials M (t-major), attention-logit
      partials, forget-gate LN stat partials.
  X3 [128, 8] f32: iou output chunk + f*cells chunk (t-major per chunk).

A tiny dummy AllGather collective (nothing consumes it) forces the runtime
to co-schedule the 8 cores; without it core launches stagger by ~1.4ms.

All full-H LayerNorms run as single gpsimd (Q7) layernorm instructions.
tanh is computed as 2*sigmoid(2x)-1 so the scalar engine only ever needs the
Sigmoid, Exp and Sqrt tables, preloaded/sequenced off the critical path.

Matmul operands are bf16; accumulation and gate math fp32.
"""

import sys

for _p in ("/opt/trn_rl_repo",):
    if _p not in sys.path:
        sys.path.insert(0, _p)

import ml_dtypes
import numpy as np

import concourse.bacc as bacc
from concourse.library_config import proxy as _proxy_lib
import concourse.bass as bass
import concourse.mybir as mybir
import concourse.tile as tile
from concourse.bass_utils import run_bass_kernel_spmd
from concourse.tile_rust import add_dep_helper

F32 = mybir.dt.float32
BF16 = mybir.dt.bfloat16
AF = mybir.ActivationFunctionType
ALU = mybir.AluOpType
NPBF = ml_dtypes.bfloat16
AXX = mybir.AxisListType.X

H = 2048
N = 32
NC = 8
S = H // NC           # 256 per-core chunk of every sharded dim
T = H // 128          # 16 tiles of 128 along a 2048 dim
KT = 32               # k-tiles along the 4096 contraction dims
EPS = 1e-5
INV_H = 1.0 / H

C1W = 528             # X1 payload cols (512 M + 3 stats + pad)
# measured device -> physical tpb mapping (probe: slots on device d hold
# data from the device with tpb(d)^j); chunk q = TPB[d] is assigned to
# device d so that device 0 (tpb 0) sees slots in chunk order.
TPB = [0, 1, 2, 3, 6, 7, 4, 5]

_CACHE = {}


def _build(dbg=False):
    nc = bacc.Bacc(None, target_bir_lowering=False, debug=False,
                   num_devices=NC)

    def din(name, shape, dt=F32):
        return nc.dram_tensor(name, list(shape), dt, kind="ExternalInput")

    # ---- per-core DRAM inputs ----
    hT = din("hT", (128, T * N), BF16)       # full hiddens k-tiles x children
    xT32 = din("xT32", (128, T * N), BF16)   # input replicated over children
    eT32 = din("eT32", (128, T * N), BF16)   # external replicated
    x1 = din("x1", (128, T), BF16)           # input t-major (iou x-half lhsT)
    hTc = din("hTc", (128, 2 * N), BF16)     # hiddens.T chunk (merge matmul)
    cells_chunk = din("cells_chunk", (N, S))
    gf_rep = din("gf_rep", (N, S))
    bf_rep = din("bf_rep", (N, S))
    wattn_rep = din("wattn_rep", (N, S))
    wat_sum = din("wat_sum", (N, 1))
    gm = din("gm", (128, T))
    bm = din("bm", (128, T))
    gi = din("gi", (128, T))
    bi = din("bi", (128, T))
    go = din("go", (128, T))
    bo = din("bo", (128, T))
    gu = din("gu", (128, T))
    bu = din("bu", (128, T))
    gc = din("gc", (128, T))
    bc = din("bc", (128, T))
    ones32 = din("ones32", (N, 1))
    onesr = din("onesr", (1, 128))
    id32 = din("id32", (N, N))
    wai = din("wai", (128, KT * S), BF16)        # W_ai^T chunk
    wf = din("wf", (128, KT * S), BF16)          # [W_fh | W_fi]^T chunk
    wmg = din("wmg", (128, 2 * H), BF16)         # W_merge^T in-chunk
    wioum = din("wioum", (128, T * 3 * S), BF16)  # W_iou^T mh-half chunk
    wioux = din("wioux", (128, T * 3 * S), BF16)  # W_iou^T x-half chunk

    out_h = nc.dram_tensor("out_h", [128, T], F32, kind="ExternalOutput")
    out_c = nc.dram_tensor("out_c", [128, T], F32, kind="ExternalOutput")
    dbg_t = {}
    if dbg:
        for nm, shp in [("d_lgf", [N, 1]), ("d_fsu", [N, 1]),
                        ("d_fss", [N, 1]), ("d_Mfull", [128, 512]),
                        ("d_ml", [128, T]), ("d_mhln", [128, T]),
                        ("d_c3", [128, 8]), ("d_iou", [1, 768]),
                        ("d_iv", [128, T]), ("d_uv", [128, T]),
                        ("d_fcv", [128, T]), ("d_iln", [128, T]),
                        ("d_cl", [128, T]), ("d_fg", [N, S]),
                        ("d_exps", [N, 1]), ("d_c1", [128, C1W]),
                        ("d_s1lg", [N, 8]), ("d_s1m", [128, 8]),
                        ("d_s3", [128, 64])]:
            dbg_t[nm] = nc.dram_tensor(nm, shp, F32, kind="ExternalOutput")

    rsem1 = [nc.alloc_semaphore(f"x1_remote_{r}") for r in range(3)]
    rsem3 = [nc.alloc_semaphore(f"x3_remote_{r}") for r in range(3)]
    lsem = nc.alloc_semaphore("x_local")

    x1_consumers = []
    x3_consumers = []
    x3_trigger_waits = []

    with tile.TileContext(nc) as tc:
        with (
            tc.tile_pool(name="sb", bufs=1) as sb,
            tc.tile_pool(name="ps", bufs=1, space="PSUM") as ps,
            tc.tile_pool(name="dram", bufs=1, space="DRAM") as dram,
        ):
            # ---- dummy co-launch collective (output unused) ----
            warm_in = dram.tile([1, 64], F32, name="warm_in")
            warm_out = dram.tile([8, 64], F32, name="warm_out")
            warm_cc = nc.gpsimd.collective_compute(
                "AllGather", ALU.bypass,
                replica_groups=[list(range(NC))],
                ins=[warm_in.opt()], outs=[warm_out.opt()])

            # ---- resident loads ----
            def load(t_dram, shape, dt=F32):
                t_sb = sb.tile(shape, dt, name=t_dram.name + "_sb")
                nc.sync.dma_start(t_sb[:], t_dram[:])
                return t_sb

            hT_sb = load(hT, [128, T, N], BF16)
            x1_sb = load(x1, [128, T], BF16)
            hTc_sb = load(hTc, [128, 2, N], BF16)

            # big weight streams issued before the small residents so the
            # attention/f/M matmuls start as early as possible
            wai_sb = sb.tile([128, KT * S], BF16, name="wai_sb")
            wf_sb = sb.tile([128, KT * S], BF16, name="wf_sb")
            wmg_sb = sb.tile([128, 2 * H], BF16, name="wmg_sb")
            wioum_sb = sb.tile([128, T * 3 * S], BF16, name="wioum_sb")
            wioux_sb = sb.tile([128, T * 3 * S], BF16, name="wioux_sb")
            wdmas = []
            for k in range(2):
                wdmas.append(nc.sync.dma_start(
                    wai_sb[:, k * 4096:(k + 1) * 4096],
                    wai[:, k * 4096:(k + 1) * 4096]))
            for k in range(2):
                wdmas.append(nc.sync.dma_start(
                    wf_sb[:, k * 4096:(k + 1) * 4096],
                    wf[:, k * 4096:(k + 1) * 4096]))
            for k in range(2):
                wdmas.append(nc.sync.dma_start(
                    wmg_sb[:, k * 2048:(k + 1) * 2048],
                    wmg[:, k * 2048:(k + 1) * 2048]))

            xT32_sb = load(xT32, [128, T, N], BF16)
            eT32_sb = load(eT32, [128, T, N], BF16)
            cells_sb = load(cells_chunk, [N, S])
            gf_sb = load(gf_rep, [N, S])
            bf_sb = load(bf_rep, [N, S])
            wat_sb = load(wattn_rep, [N, S])
            wsum_sb = load(wat_sum, [N, 1])
            gm_sb = load(gm, [128, T])
            bm_sb = load(bm, [128, T])
            gi_sb = load(gi, [128, T])
            bi_sb = load(bi, [128, T])
            go_sb = load(go, [128, T])
            bo_sb = load(bo, [128, T])
            gu_sb = load(gu, [128, T])
            bu_sb = load(bu, [128, T])
            gc_sb = load(gc, [128, T])
            bc_sb = load(bc, [128, T])
            ones32_sb = load(ones32, [N, 1])
            onesr_sb = load(onesr, [1, 128])
            id32_sb = load(id32, [N, N])

            # ---- Q7 proxy library (layernorm + remote-dma, no reloads) ----
            nc.gpsimd.load_library(_proxy_lib)
            lnw_in = sb.tile([128, 1], F32, name="lnw_in")
            lnw_out = sb.tile([128, 1], F32, name="lnw_out")
            nc.vector.memset(lnw_in[:], 1.0)
            nc.gpsimd.layernorm(lnw_out[:], lnw_in[:], eps=EPS,
                                subtract_mean=True)
            tl = sb.tile([1, 1], F32, name="tl")
            nc.vector.memset(tl[:], 0.5)
            sig_pre = nc.scalar.activation(tl[:], tl[:], AF.Sigmoid)

            # ---- tail weight streams: x-half next; mh-half is gated
            # after the X1 rounds so its HBM traffic does not contend with
            # the remote-DMA descriptor drains.
            for k in range(3):
                wdmas.append(nc.sync.dma_start(
                    wioux_sb[:, k * 4096:(k + 1) * 4096],
                    wioux[:, k * 4096:(k + 1) * 4096]))
            wioum_dmas = []
            for k in range(3):
                wioum_dmas.append(nc.sync.dma_start(
                    wioum_sb[:, k * 4096:(k + 1) * 4096],
                    wioum[:, k * 4096:(k + 1) * 4096]))
            for i in range(2, len(wdmas)):
                add_dep_helper(wdmas[i].ins, wdmas[i - 2].ins, sync=True,
                               reason="weight DMA arrival order")
            add_dep_helper(wioum_dmas[0].ins, wdmas[-1].ins, sync=True,
                           reason="mh-half after x-half")
            add_dep_helper(wioum_dmas[1].ins, wdmas[-1].ins, sync=True,
                           reason="mh-half after x-half")
            add_dep_helper(wioum_dmas[2].ins, wioum_dmas[0].ins, sync=True,
                           reason="arrival order")

            # ---- XOR-butterfly exchange buffers + desc preps ----
            # Recursive doubling over XOR partners: round r exchanges with
            # tpb^(1<<r).  One single-dest broadcast per round (1056 descs)
            # instead of 8 (8448) -- the desc drain rate (~175/us) is the
            # transport bottleneck, and dummy lanes cost as much as real ones.
            c1 = sb.tile([128, C1W], BF16, name="c1")
            b1 = [sb.tile([128, C1W], BF16, name=f"b1_{r}") for r in range(3)]
            a1 = [sb.tile([128, C1W], BF16, name=f"a1_{r}") for r in range(3)]
            g3 = sb.tile([128, NC, 8], F32, name="g3")
            c3 = g3[:, 0, :]  # own chunk lands in slot 0

            t1 = []
            prev = None
            for r in range(3):
                rd = [None] * NC
                rd[1 << r] = (0, 1 << r)
                srcap = c1[:] if r == 0 else a1[r - 1][:]
                p = nc.gpsimd.remote_dma_broadcast(
                    b1[r][:], srcap, rsem1[r], lsem, rdests=rd, queue_num=0)
                if prev is not None:
                    add_dep_helper(p.ins, prev.ins, sync=True,
                                   reason="ring FIFO order")
                t = nc.gpsimd.trigger_dma(count=None, queue_num=0)
                add_dep_helper(t.ins, p.ins, sync=True,
                               reason="trigger after its prep")
                t1.append(t)
                prev = t
            x1_last_trig = prev

            # ---- attention: ai = tanh(W_ai @ [h;e]) via 2*sig(2x)-1 ----
            ps_ai = ps.tile([N, S], F32, name="ps_ai", tag="pA")
            for kt in range(KT):
                act = hT_sb if kt < T else eT32_sb
                nc.tensor.matmul(ps_ai[:], act[:, kt % T, :],
                                 wai_sb[:, kt * S:(kt + 1) * S],
                                 start=(kt == 0), stop=(kt == KT - 1))
            ai_sig = sb.tile([N, S], F32, name="ai_sig")
            ai_act = nc.scalar.activation(ai_sig[:], ps_ai[:], AF.Sigmoid,
                                          scale=2.0)
            add_dep_helper(ai_act.ins, sig_pre.ins, sync=True,
                           reason="sigmoid table preload first")
            # logit = sum(tanh(ai)*wat) = 2*sum(sig*wat) - sum(wat)
            aw = sb.tile([N, S], F32, name="aw")
            lg0 = sb.tile([N, 1], F32, name="lg0")
            nc.vector.tensor_tensor(aw[:], ai_sig[:], wat_sb[:], op=ALU.mult)
            nc.vector.tensor_reduce(lg0[:], aw[:], AXX, ALU.add)
            lg = sb.tile([N, 1], F32, name="lg")
            nc.vector.tensor_scalar(lg[:], lg0[:], 2.0, wsum_sb[:],
                                    op0=ALU.mult, op1=ALU.subtract)
            c1w_lg = nc.vector.tensor_copy(c1[0:N, 512:513], lg[:])

            # Exp table preload after the attention sigmoid
            exp_pre = nc.scalar.activation(tl[:], tl[:], AF.Exp)
            add_dep_helper(exp_pre.ins, ai_act.ins, sync=True,
                           reason="exp preload after attention sigmoid")

            # ---- forget-gate preactivations + stat partials ----
            ps_f = ps.tile([N, S], F32, name="ps_f", tag="pB")
            for kt in range(KT):
                act = hT_sb if kt < T else xT32_sb
                nc.tensor.matmul(ps_f[:], act[:, kt % T, :],
                                 wf_sb[:, kt * S:(kt + 1) * S],
                                 start=(kt == 0), stop=(kt == KT - 1))
            f_lin = sb.tile([N, S], F32, name="f_lin")
            fsum = sb.tile([N, 1], F32, name="fsum")
            fsq = sb.tile([N, S], F32, name="fsq")
            fss = sb.tile([N, 1], F32, name="fss")
            nc.vector.tensor_copy(f_lin[:], ps_f[:])
            nc.vector.tensor_reduce(fsum[:], f_lin[:], AXX, ALU.add)
            nc.vector.scalar_tensor_tensor(fsq[:], f_lin[:], 1.0, f_lin[:],
                                           op0=ALU.mult, op1=ALU.mult,
                                           accum_out=fss[:])
            c1w_fs = nc.vector.tensor_copy(c1[0:N, 513:514], fsum[:])
            c1w_fq = nc.vector.tensor_copy(c1[0:N, 514:515], fss[:])

            # ---- speculative merge projections M[p,t,n] (in-chunk) ----
            ps_M = ps.tile([128, T, N], F32, name="ps_M", tag="pC")
            for t in range(T):
                for s in range(2):
                    nc.tensor.matmul(
                        ps_M[:, t, :],
                        wmg_sb[:, s * H + t * 128: s * H + (t + 1) * 128],
                        hTc_sb[:, s, :],
                        start=(s == 0), stop=(s == 1))
            c1w_m = nc.vector.tensor_copy(
                c1[:, 0:512].rearrange("p (t n) -> p t n", t=T), ps_M[:])
            for w in (c1w_lg, c1w_fs, c1w_fq, c1w_m):
                add_dep_helper(t1[0].ins, w.ins, sync=True,
                               reason="X1 round0 after payload writes")
            # allreduce rounds: acc_{r} = acc_{r-1} + recv_r
            accap = c1
            for r in range(3):
                ad = nc.vector.tensor_tensor(a1[r][:], accap[:], b1[r][:],
                                             op=ALU.add)
                add_dep_helper(ad.ins, t1[r].ins, sync=True,
                               reason="add after own round trigger")
                if r < 2:
                    add_dep_helper(t1[r + 1].ins, ad.ins, sync=True,
                                   reason="next round sends the new acc")
                x1_consumers.append((ad, rsem1[r]))
                accap = a1[r]
            afull = a1[2]

            # ---- iou x-half (accumulates into ps_iou) ----
            ps_iou = ps.tile([1, 3 * S], F32, name="ps_iou", tag="pD")
            for kt in range(T):
                for c0, c1_ in ((0, 512), (512, 768)):
                    nc.tensor.matmul(ps_iou[:, c0:c1_],
                                     x1_sb[:, kt:kt + 1],
                                     wioux_sb[:, kt * 768 + c0:kt * 768 + c1_],
                                     start=(kt == 0), stop=False)

            # ================= X1 consumption (afull has the sums) ========
            # softmax weights (unnormalized; scale cancels in merge LN)
            exps_c = sb.tile([N, 1], F32, name="exps_c")
            exps_act = nc.scalar.activation(exps_c[:], afull[0:N, 512:513],
                                            AF.Exp)
            add_dep_helper(exps_act.ins, exp_pre.ins, sync=True,
                           reason="exp after its preload")
            ps_er = ps.tile([1, N], F32, name="ps_er", tag="pE")
            nc.tensor.matmul(ps_er[:], exps_c[:], id32_sb[:],
                             start=True, stop=True)
            er_sb = sb.tile([1, N], F32, name="er_sb")
            nc.vector.tensor_copy(er_sb[:], ps_er[:])
            ps_eb = ps.tile([128, N], F32, name="ps_eb", tag="pF")
            nc.tensor.matmul(ps_eb[:], onesr_sb[:], er_sb[:],
                             start=True, stop=True)

            # sigmoid table back in place while the merge reduce runs
            sig_d2 = nc.scalar.activation(tl[:], tl[:], AF.Sigmoid)
            add_dep_helper(sig_d2.ins, exps_act.ins, sync=True,
                           reason="sigmoid reload after exp")

            # ml = sum_n exps_n * Mfull[:, t, n]
            eb3 = ps_eb[:].rearrange("p (one n) -> p one n",
                                     one=1).to_broadcast((128, T, N))
            msc = sb.tile([128, T, N], F32, name="msc")
            nc.vector.tensor_tensor(
                msc[:], afull[:, 0:512].rearrange("p (t n) -> p t n", t=T),
                eb3, op=ALU.mult)
            ml = sb.tile([128, T], F32, name="ml")
            nc.vector.tensor_reduce(ml[:], msc[:], AXX, ALU.add)
            mh_ln = sb.tile([128, T], F32, name="mh_ln")
            nc.gpsimd.layernorm(mh_ln[:], ml[:], gamma_ap=gm_sb[:],
                                beta_ap=bm_sb[:], eps=EPS, subtract_mean=True)
            mh_sig = sb.tile([128, T], F32, name="mh_sig")
            mh_act = nc.scalar.activation(mh_sig[:], mh_ln[:], AF.Sigmoid,
                                          scale=2.0)
            add_dep_helper(mh_act.ins, sig_d2.ins, sync=True,
                           reason="mh sigmoid after table reload")
            mh_bf = sb.tile([128, T], BF16, name="mh_bf")
            nc.vector.tensor_scalar(mh_bf[:], mh_sig[:], 2.0, -1.0,
                                    op0=ALU.mult, op1=ALU.add)

            # ---- iou mh-half (finishes ps_iou accumulation) ----
            for kt in range(T):
                for c0, c1_ in ((0, 512), (512, 768)):
                    nc.tensor.matmul(ps_iou[:, c0:c1_],
                                     mh_bf[:, kt:kt + 1],
                                     wioum_sb[:, kt * 768 + c0:kt * 768 + c1_],
                                     start=False,
                                     stop=(kt == T - 1 and c0 == 512))

            # ---- f gate -> fc = sum_n f*cells (chunk) ----
            fmean = sb.tile([N, 1], F32, name="fmean")
            fmsq = sb.tile([N, 1], F32, name="fmsq")
            fvar = sb.tile([N, 1], F32, name="fvar")
            fstd = sb.tile([N, 1], F32, name="fstd")
            frstd = sb.tile([N, 1], F32, name="frstd")
            nc.vector.tensor_scalar_mul(fmean[:], afull[0:N, 513:514],
                                        INV_H)
            nc.vector.tensor_scalar_mul(fmsq[:], afull[0:N, 514:515], INV_H)
            nc.vector.tensor_tensor(fvar[:], fmean[:], fmean[:], op=ALU.mult)
            nc.vector.tensor_sub(fvar[:], fmsq[:], fvar[:])
            nc.vector.tensor_scalar_add(fvar[:], fvar[:], EPS)
            sqrt_act = nc.scalar.activation(fstd[:], fvar[:], AF.Sqrt)
            add_dep_helper(sqrt_act.ins, mh_act.ins, sync=True,
                           reason="sqrt after mh sigmoid (table order)")
            nc.vector.reciprocal(frstd[:], fstd[:])
            ft = sb.tile([N, S], F32, name="ft")
            nc.vector.tensor_scalar(ft[:], f_lin[:], fmean[:], frstd[:],
                                    op0=ALU.subtract, op1=ALU.mult)
            nc.vector.tensor_tensor(ft[:], ft[:], gf_sb[:], op=ALU.mult)
            nc.vector.tensor_tensor(ft[:], ft[:], bf_sb[:], op=ALU.add)
            f_g = sb.tile([N, S], F32, name="f_g")
            f_act = nc.scalar.activation(f_g[:], ft[:], AF.Sigmoid)
            add_dep_helper(f_act.ins, sqrt_act.ins, sync=True,
                           reason="f sigmoid after sqrt")
            fprod = sb.tile([N, S], F32, name="fprod")
            nc.vector.tensor_tensor(fprod[:], f_g[:], cells_sb[:],
                                    op=ALU.mult)
            ps_fc = ps.tile([128, 2], F32, name="ps_fc", tag="pE")
            for s in range(2):
                nc.tensor.matmul(ps_fc[:, s:s + 1],
                                 fprod[:, s * 128:(s + 1) * 128],
                                 ones32_sb[:], start=True, stop=True)
            c3w_fc = nc.vector.tensor_copy(c3[:, 6:8], ps_fc[:])
            iou_sb = sb.tile([1, 3 * S], F32, name="iou_sb")
            nc.vector.tensor_copy(iou_sb[:], ps_iou[:])
            # reshape [1,768] -> [128,6] t-major via 6 PE transposes
            ps_tr = ps.tile([128, 6], F32, name="ps_tr", tag="pF")
            for k in range(6):
                nc.tensor.matmul(ps_tr[:, k:k + 1],
                                 iou_sb[0:1, k * 128:(k + 1) * 128],
                                 onesr_sb[0:1, 0:1], is_transpose=True,
                                 start=True, stop=True)
            c3w_iou = nc.vector.tensor_copy(c3[:, 0:6], ps_tr[:])
            # X3: warm ncfw AllGather of the [128,8] chunk (rank order)
            ag3_in = dram.tile([1, 1024], F32, name="ag3_in")
            ag3_out = dram.tile([NC, 1024], F32, name="ag3_out")
            nc.sync.dma_start(
                ag3_in[0, :].rearrange("(p c) -> p c", p=128), c3)
            nc.gpsimd.collective_compute(
                "AllGather", ALU.bypass,
                replica_groups=[list(range(NC))],
                ins=[ag3_in.opt()], outs=[ag3_out.opt()])
            nc.sync.dma_start(
                g3[:], ag3_out[:, :].rearrange("s (p c) -> p s c", p=128))

            # ================= X3 consumption: final gates =================
            vec = {}
            for idx, nm in ((0, "iv"), (1, "ov"), (2, "uv"), (3, "fcv")):
                vt = sb.tile([128, T], F32, name=nm)
                ci = nc.vector.tensor_copy(
                    vt[:].rearrange("p (s d) -> p s d", s=NC),
                    g3[:, :, 2 * idx:2 * idx + 2])
                x3_consumers.append(ci)
                vec[nm] = vt

            i_ln = sb.tile([128, T], F32, name="i_ln")
            o_ln = sb.tile([128, T], F32, name="o_ln")
            u_ln = sb.tile([128, T], F32, name="u_ln")
            nc.gpsimd.layernorm(i_ln[:], vec["iv"][:], gamma_ap=gi_sb[:],
                                beta_ap=bi_sb[:], eps=EPS, subtract_mean=True)
            nc.gpsimd.layernorm(o_ln[:], vec["ov"][:], gamma_ap=go_sb[:],
                                beta_ap=bo_sb[:], eps=EPS, subtract_mean=True)
            nc.gpsimd.layernorm(u_ln[:], vec["uv"][:], gamma_ap=gu_sb[:],
                                beta_ap=bu_sb[:], eps=EPS, subtract_mean=True)
            i_g = sb.tile([128, T], F32, name="i_g")
            o_g = sb.tile([128, T], F32, name="o_g")
            u_s = sb.tile([128, T], F32, name="u_s")
            nc.scalar.activation(i_g[:], i_ln[:], AF.Sigmoid)
            nc.scalar.activation(o_g[:], o_ln[:], AF.Sigmoid)
            nc.scalar.activation(u_s[:], u_ln[:], AF.Sigmoid, scale=2.0)
            u_g = sb.tile([128, T], F32, name="u_g")
            nc.vector.tensor_scalar(u_g[:], u_s[:], 2.0, -1.0,
                                    op0=ALU.mult, op1=ALU.add)
            cl = sb.tile([128, T], F32, name="cl")
            nc.vector.tensor_tensor(cl[:], i_g[:], u_g[:], op=ALU.mult)
            nc.vector.tensor_tensor(cl[:], cl[:], vec["fcv"][:], op=ALU.add)
            new_c = sb.tile([128, T], F32, name="new_c")
            nc.gpsimd.layernorm(new_c[:], cl[:], gamma_ap=gc_sb[:],
                                beta_ap=bc_sb[:], eps=EPS, subtract_mean=True)
            th_s = sb.tile([128, T], F32, name="th_s")
            nc.scalar.activation(th_s[:], new_c[:], AF.Sigmoid, scale=2.0)
            th = sb.tile([128, T], F32, name="th")
            nc.vector.tensor_scalar(th[:], th_s[:], 2.0, -1.0,
                                    op0=ALU.mult, op1=ALU.add)
            new_h = sb.tile([128, T], F32, name="new_h")
            nc.vector.tensor_tensor(new_h[:], o_g[:], th[:], op=ALU.mult)

            nc.sync.dma_start(out_c[:], new_c[:])
            nc.sync.dma_start(out_h[:], new_h[:])
            if dbg:
                for nm, src_t in [("d_ml", ml), ("d_mhln", mh_ln),
                                  ("d_c3", c3), ("d_iou", iou_sb),
                                  ("d_iv", vec["iv"]), ("d_uv", vec["uv"]),
                                  ("d_fcv", vec["fcv"]), ("d_iln", i_ln),
                                  ("d_cl", cl), ("d_fg", f_g),
                                  ("d_exps", exps_c)]:
                    dd = sb.tile(list(dbg_t[nm].shape), F32, name=nm + "_d")
                    nc.vector.tensor_copy(dd[:], src_t[:])
                    nc.sync.dma_start(dbg_t[nm][:], dd[:])
                for nm, ap in [("d_c1", c1[:]),
                               ("d_Mfull", afull[:, 0:512]),
                               ("d_lgf", afull[0:N, 512:513]),
                               ("d_fsu", afull[0:N, 513:514]),
                               ("d_fss", afull[0:N, 514:515]),
                               ("d_s3", g3[:].rearrange("p s f -> p (s f)"))]:
                    dd = sb.tile(list(dbg_t[nm].shape), F32, name=nm + "_d")
                    di = nc.vector.tensor_copy(dd[:], ap)
                    add_dep_helper(di.ins, x3_consumers[0].ins, sync=True,
                                   reason="dbg after X3 wait")
                    nc.sync.dma_start(dbg_t[nm][:], dd[:])


    # Remote-arrival waits, invisible to the single-core scheduling sim:
    # patched after tile scheduling, split into event semaphores at compile.
    for ad, sem in x1_consumers:
        bass.BassInstruction(ad.ins).wait_op(sem, 2, "sem-ge", check=False)


    nc.compile()
    return nc


def _tmaj(v):
    """[2048] vector -> [128,16] t-major sbuf image (sb[p,t] = v[t*128+p])."""
    return np.ascontiguousarray(v.reshape(T, 128).T)


def _ktiles(wT, cols):
    """wT: [K_in, out_cols] -> [128, (K_in/128)*cols] partition-major pack."""
    k_in = wT.shape[0]
    return np.ascontiguousarray(
        wT.reshape(k_in // 128, 128, cols).transpose(1, 0, 2).reshape(
            128, (k_in // 128) * cols))


def kernel(input, hiddens, cells, external,
           W_ai, W_attn, W_merge, W_iou, W_fi, W_fh,
           g_merge, b_merge, g_f, b_f, g_i, b_i, g_o, b_o, g_u, b_u,
           g_c, b_c):
    key = ("nc", bool(_CACHE.get("dbg")))
    if key not in _CACHE:
        _CACHE[key] = _build(bool(_CACHE.get("dbg")))
    nc = _CACHE[key]

    f32 = np.float32
    input = np.asarray(input, f32)
    hiddens = np.asarray(hiddens, f32)
    cells = np.asarray(cells, f32)
    external = np.asarray(external, f32)

    hTt = _ktiles(np.ascontiguousarray(hiddens.T), N).astype(NPBF)
    xT32 = _ktiles(np.tile(input[:, None], (1, N)), N).astype(NPBF)
    eT32 = _ktiles(np.tile(external[:, None], (1, N)), N).astype(NPBF)
    x1v = _tmaj(input).astype(NPBF)

    com = {
        "hT": hTt, "xT32": xT32, "eT32": eT32, "x1": x1v,
        "gm": _tmaj(g_merge), "bm": _tmaj(b_merge),
        "gi": _tmaj(g_i), "bi": _tmaj(b_i),
        "go": _tmaj(g_o), "bo": _tmaj(b_o),
        "gu": _tmaj(g_u), "bu": _tmaj(b_u),
        "gc": _tmaj(g_c), "bc": _tmaj(b_c),
        "ones32": np.ones((N, 1), f32),
        "onesr": np.ones((1, 128), f32),
        "id32": np.eye(N, dtype=f32),
    }

    Wf_cat = np.concatenate([W_fh, W_fi], axis=1)              # [H, 4096]
    in_maps = []
    for d in range(NC):
        q = d  # ncfw AllGather orders X3 slots by rank
        r = slice(q * S, (q + 1) * S)
        iou_rows = np.concatenate(
            [W_iou[g * H + q * S:g * H + (q + 1) * S, :] for g in range(3)],
            axis=0)                                            # [768, 4096]
        m = dict(com)
        m.update({
            "hTc": np.ascontiguousarray(
                hiddens.T[q * S:(q + 1) * S].reshape(2, 128, N)
                .transpose(1, 0, 2).reshape(128, 2 * N)).astype(NPBF),
            "cells_chunk": np.ascontiguousarray(cells[:, r]),
            "gf_rep": np.tile(g_f[r], (N, 1)),
            "bf_rep": np.tile(b_f[r], (N, 1)),
            "wattn_rep": np.tile(W_attn[0, r], (N, 1)),
            "wat_sum": np.full((N, 1), W_attn[0, r].sum(), f32),
            "wf": _ktiles(np.ascontiguousarray(Wf_cat[r].T), S).astype(NPBF),
            "wai": _ktiles(np.ascontiguousarray(W_ai[r].T), S).astype(NPBF),
            "wmg": _ktiles(np.ascontiguousarray(W_merge[:, r].T),
                           H).astype(NPBF),
            "wioum": _ktiles(np.ascontiguousarray(iou_rows[:, H:].T),
                             3 * S).astype(NPBF),
            "wioux": _ktiles(np.ascontiguousarray(iou_rows[:, :H].T),
                             3 * S).astype(NPBF),
        })
        in_maps.append({k: (np.ascontiguousarray(v) if v.dtype == NPBF
                            else np.ascontiguousarray(v, f32))
                        for k, v in m.items()})

    res = run_bass_kernel_spmd(nc, in_maps, core_ids=list(range(NC)))
    _CACHE["last_results"] = res
    r0 = res.results[0]
    new_h = np.ascontiguousarray(r0["out_h"].T).reshape(H)
    new_c = np.ascontiguousarray(r0["out_c"].T).reshape(H)
    return new_h, new_c


# revision 18
# speedup vs baseline: 1.3353x; 1.3353x over previous
"""AttentiveChildSumTreeLSTMCell on 8 Trainium2 NeuronCores.

Structure (one NEFF, SPMD on 8 cores):
  * X1 cross-core allreduce via XOR-butterfly peer-to-peer SWDGE remote DMA
    (3 rounds, partner tpb^2^r): merge-projection partials M, attention
    logit partials, forget-gate LN stat partials -- one [128,528] bf16
    payload.  Bypasses ncfw (cold-start ~60us) entirely.
  * X3 final AllGather of the per-core iou/fc chunk via a warm ncfw
    collective (a tiny co-launch AllGather fires first; NEFFs without any
    collective get launched ~1.4ms staggered).
  * All full-H LayerNorms are single Q7 gpsimd.layernorm instructions; the
    `proxy` library holds layernorm + remote-dma so there is no mid-kernel
    IRAM reload.  tanh(x) = 2*sigmoid(2x)-1 keeps the scalar engine on the
    Sigmoid table; Exp/Sqrt loads are sequenced off the critical path.
  * Inputs are packed into 3 blobs per core (weights bf16 / activations
    bf16 / params f32) to minimize per-device dispatch overhead.

Matmul operands are bf16; accumulation and gate math fp32.
"""

import sys

for _p in ("/opt/trn_rl_repo",):
    if _p not in sys.path:
        sys.path.insert(0, _p)

import ml_dtypes
import numpy as np

import concourse.bacc as bacc
import concourse.bass as bass
import concourse.mybir as mybir
import concourse.tile as tile
from concourse.bass_utils import run_bass_kernel_spmd
from concourse.library_config import proxy as _proxy_lib
from concourse.tile_rust import add_dep_helper

F32 = mybir.dt.float32
BF16 = mybir.dt.bfloat16
AF = mybir.ActivationFunctionType
ALU = mybir.AluOpType
NPBF = ml_dtypes.bfloat16
AXX = mybir.AxisListType.X

H = 2048
N = 32
NC = 8
S = H // NC           # 256 per-core chunk of every sharded dim
T = H // 128          # 16 tiles of 128 along a 2048 dim
KT = 32               # k-tiles along the 4096 contraction dims
EPS = 1e-5
INV_H = 1.0 / H

C1W = 528             # X1 payload cols (512 M + 3 stats + pad)

# weight blob column offsets (bf16 cols)
W_AI, W_F, W_MG, W_IOM, W_IOX = 0, 8192, 16384, 20480, 32768
WB_COLS = 45056
# activation blob (bf16 cols): hT | xT32 | eT32 | x1 | hTc
B_HT, B_XT, B_ET, B_X1, B_HTC = 0, 512, 1024, 1536, 1552
BB_COLS = 1616
# param blob (f32 cols)
P_GB, P_ID, P_CELL, P_GF, P_BF, P_WAT, P_WS = 0, 160, 192, 448, 704, 960, 1216
PB_COLS = 1217

_CACHE = {}


def _build(dbg=False):
    nc = bacc.Bacc(None, target_bir_lowering=False, debug=False,
                   num_devices=NC)

    wb = nc.dram_tensor("wb", [128, WB_COLS], BF16, kind="ExternalInput")
    bb = nc.dram_tensor("bb", [128, BB_COLS], BF16, kind="ExternalInput")
    pb = nc.dram_tensor("pb", [128, PB_COLS], F32, kind="ExternalInput")
    out_h = nc.dram_tensor("out_h", [128, T], F32, kind="ExternalOutput")
    out_c = nc.dram_tensor("out_c", [128, T], F32, kind="ExternalOutput")
    dbg_t = {}
    if dbg:
        for nm, shp in [("d_ml", [128, T]), ("d_mhln", [128, T]),
                        ("d_iou", [1, 768]), ("d_iv", [128, T]),
                        ("d_uv", [128, T]), ("d_fcv", [128, T]),
                        ("d_iln", [128, T]), ("d_cl", [128, T]),
                        ("d_fg", [N, S]), ("d_exps", [N, 1]),
                        ("d_c1", [128, C1W]), ("d_Mfull", [128, 512]),
                        ("d_lgf", [N, 1]), ("d_fsu", [N, 1]),
                        ("d_fss", [N, 1]), ("d_s3", [128, 64])]:
            dbg_t[nm] = nc.dram_tensor(nm, shp, F32, kind="ExternalOutput")

    rsem1 = [nc.alloc_semaphore(f"x1_remote_{r}") for r in range(3)]
    lsem = nc.alloc_semaphore("x_local")
    x1_consumers = []

    with tile.TileContext(nc) as tc:
        with (
            tc.tile_pool(name="sb", bufs=1) as sb,
            tc.tile_pool(name="ps", bufs=1, space="PSUM") as ps,
            tc.tile_pool(name="dram", bufs=1, space="DRAM") as dram,
        ):
            # ---- co-launch / ncfw warm-up collective (output unused) ----
            warm_in = dram.tile([1, 64], F32, name="warm_in")
            warm_out = dram.tile([8, 64], F32, name="warm_out")
            nc.gpsimd.collective_compute(
                "AllGather", ALU.bypass,
                replica_groups=[list(range(NC))],
                ins=[warm_in.opt()], outs=[warm_out.opt()])

            # ---- input loads: activations, weights (ordered), params ----
            bb_sb = sb.tile([128, BB_COLS], BF16, name="bb_sb")
            nc.sync.dma_start(bb_sb[:], bb[:])
            wb_sb = sb.tile([128, WB_COLS], BF16, name="wb_sb")
            wdmas = []
            # arrival order: wai, wf, wmg, wioux, wioum (in ~1MB chunks)
            ranges = ([(W_AI + k * 4096, W_AI + (k + 1) * 4096)
                       for k in range(2)]
                      + [(W_F + k * 4096, W_F + (k + 1) * 4096)
                         for k in range(2)]
                      + [(W_MG + k * 2048, W_MG + (k + 1) * 2048)
                         for k in range(2)]
                      + [(W_IOX + k * 4096, W_IOX + (k + 1) * 4096)
                         for k in range(3)]
                      + [(W_IOM + k * 4096, W_IOM + (k + 1) * 4096)
                         for k in range(3)])
            for a, b in ranges:
                wdmas.append(nc.sync.dma_start(wb_sb[:, a:b], wb[:, a:b]))
            for i in range(2, len(wdmas)):
                add_dep_helper(wdmas[i].ins, wdmas[i - 2].ins, sync=True,
                               reason="weight DMA arrival order")
            pb_sb = sb.tile([128, PB_COLS], F32, name="pb_sb")
            nc.sync.dma_start(pb_sb[:], pb[:])

            # views into the blobs
            hT_sb = bb_sb[:, B_HT:B_XT].rearrange("p (t n) -> p t n", t=T)
            xT32_sb = bb_sb[:, B_XT:B_ET].rearrange("p (t n) -> p t n", t=T)
            eT32_sb = bb_sb[:, B_ET:B_X1].rearrange("p (t n) -> p t n", t=T)
            x1_sb = bb_sb[:, B_X1:B_X1 + T]
            hTc_sb = bb_sb[:, B_HTC:B_HTC + 2 * N].rearrange(
                "p (s n) -> p s n", s=2)
            (gm_sb, bm_sb, gi_sb, bi_sb, go_sb, bo_sb, gu_sb, bu_sb,
             gc_sb, bc_sb) = (pb_sb[:, P_GB + k * T:P_GB + (k + 1) * T]
                              for k in range(10))
            id32_sb = pb_sb[0:N, P_ID:P_ID + N]
            cells_sb = pb_sb[0:N, P_CELL:P_CELL + S]
            gf_sb = pb_sb[0:N, P_GF:P_GF + S]
            bf_sb = pb_sb[0:N, P_BF:P_BF + S]
            wat_sb = pb_sb[0:N, P_WAT:P_WAT + S]
            wsum_sb = pb_sb[0:N, P_WS:P_WS + 1]

            ones32_sb = sb.tile([N, 1], F32, name="ones32_sb")
            nc.vector.memset(ones32_sb[:], 1.0)
            onesr_sb = sb.tile([1, 128], F32, name="onesr_sb")
            nc.vector.memset(onesr_sb[:], 1.0)

            # ---- Q7 proxy library (layernorm + remote-dma, no reloads) ----
            nc.gpsimd.load_library(_proxy_lib)
            lnw_in = sb.tile([128, 1], F32, name="lnw_in")
            lnw_out = sb.tile([128, 1], F32, name="lnw_out")
            nc.vector.memset(lnw_in[:], 1.0)
            nc.gpsimd.layernorm(lnw_out[:], lnw_in[:], eps=EPS,
                                subtract_mean=True)
            tl = sb.tile([1, 1], F32, name="tl")
            nc.vector.memset(tl[:], 0.5)
            sig_pre = nc.scalar.activation(tl[:], tl[:], AF.Sigmoid)

            # ---- X1 XOR-butterfly buffers + round preps/triggers ----
            c1 = sb.tile([128, C1W], BF16, name="c1")
            b1 = [sb.tile([128, C1W], BF16, name=f"b1_{r}") for r in range(3)]
            a1 = [sb.tile([128, C1W], BF16, name=f"a1_{r}") for r in range(3)]
            g3 = sb.tile([128, NC, 8], F32, name="g3")
            c3 = g3[:, 0, :]  # own chunk lands in slot 0

            t1 = []
            prev = None
            for r in range(3):
                rd = [None] * NC
                rd[1 << r] = (0, 1 << r)
                srcap = c1[:] if r == 0 else a1[r - 1][:]
                p = nc.gpsimd.remote_dma_broadcast(
                    b1[r][:], srcap, rsem1[r], lsem, rdests=rd, queue_num=0)
                if prev is not None:
                    add_dep_helper(p.ins, prev.ins, sync=True,
                                   reason="ring FIFO order")
                t = nc.gpsimd.trigger_dma(count=None, queue_num=0)
                add_dep_helper(t.ins, p.ins, sync=True,
                               reason="trigger after its prep")
                t1.append(t)
                prev = t

            # ---- attention: ai = tanh(W_ai @ [h;e]) via 2*sig(2x)-1 ----
            ps_ai = ps.tile([N, S], F32, name="ps_ai", tag="pA")
            for kt in range(KT):
                act = hT_sb if kt < T else eT32_sb
                nc.tensor.matmul(ps_ai[:], act[:, kt % T, :],
                                 wb_sb[:, W_AI + kt * S:W_AI + (kt + 1) * S],
                                 start=(kt == 0), stop=(kt == KT - 1))
            ai_sig = sb.tile([N, S], F32, name="ai_sig")
            ai_act = nc.scalar.activation(ai_sig[:], ps_ai[:], AF.Sigmoid,
                                          scale=2.0)
            add_dep_helper(ai_act.ins, sig_pre.ins, sync=True,
                           reason="sigmoid table preload first")
            # logit = sum(tanh(ai)*wat) = 2*sum(sig*wat) - sum(wat)
            aw = sb.tile([N, S], F32, name="aw")
            lg0 = sb.tile([N, 1], F32, name="lg0")
            nc.vector.tensor_tensor(aw[:], ai_sig[:], wat_sb, op=ALU.mult)
            nc.vector.tensor_reduce(lg0[:], aw[:], AXX, ALU.add)
            lg = sb.tile([N, 1], F32, name="lg")
            nc.vector.tensor_scalar(lg[:], lg0[:], 2.0, wsum_sb,
                                    op0=ALU.mult, op1=ALU.subtract)
            c1w_lg = nc.vector.tensor_copy(c1[0:N, 512:513], lg[:])

            # Exp table preload after the attention sigmoid
            exp_pre = nc.scalar.activation(tl[:], tl[:], AF.Exp)
            add_dep_helper(exp_pre.ins, ai_act.ins, sync=True,
                           reason="exp preload after attention sigmoid")

            # ---- forget-gate preactivations + stat partials ----
            ps_f = ps.tile([N, S], F32, name="ps_f", tag="pB")
            for kt in range(KT):
                act = hT_sb if kt < T else xT32_sb
                nc.tensor.matmul(ps_f[:], act[:, kt % T, :],
                                 wb_sb[:, W_F + kt * S:W_F + (kt + 1) * S],
                                 start=(kt == 0), stop=(kt == KT - 1))
            f_lin = sb.tile([N, S], F32, name="f_lin")
            fsum = sb.tile([N, 1], F32, name="fsum")
            fsq = sb.tile([N, S], F32, name="fsq")
            fss = sb.tile([N, 1], F32, name="fss")
            nc.vector.tensor_copy(f_lin[:], ps_f[:])
            nc.vector.tensor_reduce(fsum[:], f_lin[:], AXX, ALU.add)
            nc.vector.scalar_tensor_tensor(fsq[:], f_lin[:], 1.0, f_lin[:],
                                           op0=ALU.mult, op1=ALU.mult,
                                           accum_out=fss[:])
            c1w_fs = nc.vector.tensor_copy(c1[0:N, 513:514], fsum[:])
            c1w_fq = nc.vector.tensor_copy(c1[0:N, 514:515], fss[:])

            # ---- speculative merge projections M[p,t,n] (in-chunk) ----
            ps_M = ps.tile([128, T, N], F32, name="ps_M", tag="pC")
            for t in range(T):
                for s in range(2):
                    nc.tensor.matmul(
                        ps_M[:, t, :],
                        wb_sb[:, W_MG + s * H + t * 128:
                              W_MG + s * H + (t + 1) * 128],
                        hTc_sb[:, s, :],
                        start=(s == 0), stop=(s == 1))
            c1w_m = nc.vector.tensor_copy(
                c1[:, 0:512].rearrange("p (t n) -> p t n", t=T), ps_M[:])
            for w in (c1w_lg, c1w_fs, c1w_fq, c1w_m):
                add_dep_helper(t1[0].ins, w.ins, sync=True,
                               reason="X1 round0 after payload writes")
            # allreduce rounds: acc_{r} = acc_{r-1} + recv_r
            accap = c1
            for r in range(3):
                ad = nc.vector.tensor_tensor(a1[r][:], accap[:], b1[r][:],
                                             op=ALU.add)
                add_dep_helper(ad.ins, t1[r].ins, sync=True,
                               reason="add after own round trigger")
                if r < 2:
                    add_dep_helper(t1[r + 1].ins, ad.ins, sync=True,
                                   reason="next round sends the new acc")
                x1_consumers.append((ad, rsem1[r]))
                accap = a1[r]
            afull = a1[2]

            # ---- iou x-half (accumulates into ps_iou) ----
            ps_iou = ps.tile([1, 3 * S], F32, name="ps_iou", tag="pD")
            for kt in range(T):
                for c0, c1_ in ((0, 512), (512, 768)):
                    nc.tensor.matmul(
                        ps_iou[:, c0:c1_], x1_sb[:, kt:kt + 1],
                        wb_sb[:, W_IOX + kt * 768 + c0:
                              W_IOX + kt * 768 + c1_],
                        start=(kt == 0), stop=False)

            # ================= X1 consumption (afull has the sums) ========
            exps_c = sb.tile([N, 1], F32, name="exps_c")
            exps_act = nc.scalar.activation(exps_c[:], afull[0:N, 512:513],
                                            AF.Exp)
            add_dep_helper(exps_act.ins, exp_pre.ins, sync=True,
                           reason="exp after its preload")
            ps_er = ps.tile([1, N], F32, name="ps_er", tag="pE")
            nc.tensor.matmul(ps_er[:], exps_c[:], id32_sb,
                             start=True, stop=True)
            er_sb = sb.tile([1, N], F32, name="er_sb")
            nc.vector.tensor_copy(er_sb[:], ps_er[:])
            ps_eb = ps.tile([128, N], F32, name="ps_eb", tag="pF")
            nc.tensor.matmul(ps_eb[:], onesr_sb[:], er_sb[:],
                             start=True, stop=True)

            # sigmoid table back in place while the merge reduce runs
            sig_d2 = nc.scalar.activation(tl[:], tl[:], AF.Sigmoid)
            add_dep_helper(sig_d2.ins, exps_act.ins, sync=True,
                           reason="sigmoid reload after exp")

            # ml = sum_n exps_n * Mfull[:, t, n]
            eb3 = ps_eb[:].rearrange("p (one n) -> p one n",
                                     one=1).to_broadcast((128, T, N))
            msc = sb.tile([128, T, N], F32, name="msc")
            nc.vector.tensor_tensor(
                msc[:], afull[:, 0:512].rearrange("p (t n) -> p t n", t=T),
                eb3, op=ALU.mult)
            ml = sb.tile([128, T], F32, name="ml")
            nc.vector.tensor_reduce(ml[:], msc[:], AXX, ALU.add)
            mh_ln = sb.tile([128, T], F32, name="mh_ln")
            nc.gpsimd.layernorm(mh_ln[:], ml[:], gamma_ap=gm_sb,
                                beta_ap=bm_sb, eps=EPS, subtract_mean=True)
            mh_sig = sb.tile([128, T], F32, name="mh_sig")
            mh_act = nc.scalar.activation(mh_sig[:], mh_ln[:], AF.Sigmoid,
                                          scale=2.0)
            add_dep_helper(mh_act.ins, sig_d2.ins, sync=True,
                           reason="mh sigmoid after table reload")
            mh_bf = sb.tile([128, T], BF16, name="mh_bf")
            nc.vector.tensor_scalar(mh_bf[:], mh_sig[:], 2.0, -1.0,
                                    op0=ALU.mult, op1=ALU.add)

            # ---- iou mh-half (finishes ps_iou accumulation) ----
            for kt in range(T):
                for c0, c1_ in ((0, 512), (512, 768)):
                    nc.tensor.matmul(
                        ps_iou[:, c0:c1_], mh_bf[:, kt:kt + 1],
                        wb_sb[:, W_IOM + kt * 768 + c0:
                              W_IOM + kt * 768 + c1_],
                        start=False, stop=(kt == T - 1 and c0 == 512))

            # ---- f gate -> fc = sum_n f*cells (chunk) ----
            fmean = sb.tile([N, 1], F32, name="fmean")
            fmsq = sb.tile([N, 1], F32, name="fmsq")
            fvar = sb.tile([N, 1], F32, name="fvar")
            fstd = sb.tile([N, 1], F32, name="fstd")
            frstd = sb.tile([N, 1], F32, name="frstd")
            nc.vector.tensor_scalar_mul(fmean[:], afull[0:N, 513:514], INV_H)
            nc.vector.tensor_scalar_mul(fmsq[:], afull[0:N, 514:515], INV_H)
            nc.vector.tensor_tensor(fvar[:], fmean[:], fmean[:], op=ALU.mult)
            nc.vector.tensor_sub(fvar[:], fmsq[:], fvar[:])
            nc.vector.tensor_scalar_add(fvar[:], fvar[:], EPS)
            sqrt_act = nc.scalar.activation(fstd[:], fvar[:], AF.Sqrt)
            add_dep_helper(sqrt_act.ins, mh_act.ins, sync=True,
                           reason="sqrt after mh sigmoid (table order)")
            nc.vector.reciprocal(frstd[:], fstd[:])
            ft = sb.tile([N, S], F32, name="ft")
            nc.vector.tensor_scalar(ft[:], f_lin[:], fmean[:], frstd[:],
                                    op0=ALU.subtract, op1=ALU.mult)
            nc.vector.tensor_tensor(ft[:], ft[:], gf_sb, op=ALU.mult)
            nc.vector.tensor_tensor(ft[:], ft[:], bf_sb, op=ALU.add)
            f_g = sb.tile([N, S], F32, name="f_g")
            f_act = nc.scalar.activation(f_g[:], ft[:], AF.Sigmoid)
            add_dep_helper(f_act.ins, sqrt_act.ins, sync=True,
                           reason="f sigmoid after sqrt")
            fprod = sb.tile([N, S], F32, name="fprod")
            nc.vector.tensor_tensor(fprod[:], f_g[:], cells_sb, op=ALU.mult)
            ps_fc = ps.tile([128, 2], F32, name="ps_fc", tag="pE")
            for s in range(2):
                nc.tensor.matmul(ps_fc[:, s:s + 1],
                                 fprod[:, s * 128:(s + 1) * 128],
                                 ones32_sb[:], start=True, stop=True)
            c3w_fc = nc.vector.tensor_copy(c3[:, 6:8], ps_fc[:])

            iou_sb = sb.tile([1, 3 * S], F32, name="iou_sb")
            nc.vector.tensor_copy(iou_sb[:], ps_iou[:])
            # reshape [1,768] -> [128,6] t-major via 6 PE transposes
            ps_tr = ps.tile([128, 6], F32, name="ps_tr", tag="pF")
            for k in range(6):
                nc.tensor.matmul(ps_tr[:, k:k + 1],
                                 iou_sb[0:1, k * 128:(k + 1) * 128],
                                 onesr_sb[0:1, 0:1], is_transpose=True,
                                 start=True, stop=True)
            c3w_iou = nc.vector.tensor_copy(c3[:, 0:6], ps_tr[:])
            # X3: warm ncfw AllGather of the [128,8] chunk (rank order)
            ag3_in = dram.tile([1, 1024], F32, name="ag3_in")
            ag3_out = dram.tile([NC, 1024], F32, name="ag3_out")
            nc.sync.dma_start(
                ag3_in[0, :].rearrange("(p c) -> p c", p=128), c3)
            nc.gpsimd.collective_compute(
                "AllGather", ALU.bypass,
                replica_groups=[list(range(NC))],
                ins=[ag3_in.opt()], outs=[ag3_out.opt()])
            nc.sync.dma_start(
                g3[:], ag3_out[:, :].rearrange("s (p c) -> p s c", p=128))

            # ================= X3 consumption: final gates =================
            vec = {}
            for idx, nm in ((0, "iv"), (1, "ov"), (2, "uv"), (3, "fcv")):
                vt = sb.tile([128, T], F32, name=nm)
                nc.vector.tensor_copy(
                    vt[:].rearrange("p (s d) -> p s d", s=NC),
                    g3[:, :, 2 * idx:2 * idx + 2])
                vec[nm] = vt

            i_ln = sb.tile([128, T], F32, name="i_ln")
            o_ln = sb.tile([128, T], F32, name="o_ln")
            u_ln = sb.tile([128, T], F32, name="u_ln")
            nc.gpsimd.layernorm(i_ln[:], vec["iv"][:], gamma_ap=gi_sb,
                                beta_ap=bi_sb, eps=EPS, subtract_mean=True)
            nc.gpsimd.layernorm(o_ln[:], vec["ov"][:], gamma_ap=go_sb,
                                beta_ap=bo_sb, eps=EPS, subtract_mean=True)
            nc.gpsimd.layernorm(u_ln[:], vec["uv"][:], gamma_ap=gu_sb,
                                beta_ap=bu_sb, eps=EPS, subtract_mean=True)
            i_g = sb.tile([128, T], F32, name="i_g")
            o_g = sb.tile([128, T], F32, name="o_g")
            u_s = sb.tile([128, T], F32, name="u_s")
            nc.scalar.activation(i_g[:], i_ln[:], AF.Sigmoid)
            nc.scalar.activation(o_g[:], o_ln[:], AF.Sigmoid)
            nc.scalar.activation(u_s[:], u_ln[:], AF.Sigmoid, scale=2.0)
            u_g = sb.tile([128, T], F32, name="u_g")
            nc.vector.tensor_scalar(u_g[:], u_s[:], 2.0, -1.0,
                                    op0=ALU.mult, op1=ALU.add)
            cl = sb.tile([128, T], F32, name="cl")
            nc.vector.tensor_tensor(cl[:], i_g[:], u_g[:], op=ALU.mult)
            nc.vector.tensor_tensor(cl[:], cl[:], vec["fcv"][:], op=ALU.add)
            new_c = sb.tile([128, T], F32, name="new_c")
            nc.gpsimd.layernorm(new_c[:], cl[:], gamma_ap=gc_sb,
                                beta_ap=bc_sb, eps=EPS, subtract_mean=True)
            th_s = sb.tile([128, T], F32, name="th_s")
            nc.scalar.activation(th_s[:], new_c[:], AF.Sigmoid, scale=2.0)
            th = sb.tile([128, T], F32, name="th")
            nc.vector.tensor_scalar(th[:], th_s[:], 2.0, -1.0,
                                    op0=ALU.mult, op1=ALU.add)
            new_h = sb.tile([128, T], F32, name="new_h")
            nc.vector.tensor_tensor(new_h[:], o_g[:], th[:], op=ALU.mult)

            nc.sync.dma_start(out_c[:], new_c[:])
            nc.sync.dma_start(out_h[:], new_h[:])
            if dbg:
                for nm, src_t in [("d_ml", ml[:]), ("d_mhln", mh_ln[:]),
                                  ("d_iou", iou_sb[:]),
                                  ("d_iv", vec["iv"][:]),
                                  ("d_uv", vec["uv"][:]),
                                  ("d_fcv", vec["fcv"][:]),
                                  ("d_iln", i_ln[:]), ("d_cl", cl[:]),
                                  ("d_fg", f_g[:]), ("d_exps", exps_c[:]),
                                  ("d_c1", c1[:]),
                                  ("d_Mfull", afull[:, 0:512]),
                                  ("d_lgf", afull[0:N, 512:513]),
                                  ("d_fsu", afull[0:N, 513:514]),
                                  ("d_fss", afull[0:N, 514:515]),
                                  ("d_s3", g3[:].rearrange(
                                      "p s f -> p (s f)"))]:
                    dd = sb.tile(list(dbg_t[nm].shape), F32, name=nm + "_d")
                    nc.vector.tensor_copy(dd[:], src_t)
                    nc.sync.dma_start(dbg_t[nm][:], dd[:])

    # Remote-arrival waits, invisible to the single-core scheduling sim:
    # patched after tile scheduling, split into event semaphores at compile.
    for ad, sem in x1_consumers:
        bass.BassInstruction(ad.ins).wait_op(sem, 2, "sem-ge", check=False)

    nc.compile()
    return nc


def _tmaj(v):
    """[2048] vector -> [128,16] t-major sbuf image (sb[p,t] = v[t*128+p])."""
    return np.ascontiguousarray(v.reshape(T, 128).T)


def _ktiles(wT, cols):
    """wT: [K_in, out_cols] -> [128, (K_in/128)*cols] partition-major pack."""
    k_in = wT.shape[0]
    return np.ascontiguousarray(
        wT.reshape(k_in // 128, 128, cols).transpose(1, 0, 2).reshape(
            128, (k_in // 128) * cols))


def kernel(input, hiddens, cells, external,
           W_ai, W_attn, W_merge, W_iou, W_fi, W_fh,
           g_merge, b_merge, g_f, b_f, g_i, b_i, g_o, b_o, g_u, b_u,
           g_c, b_c):
    key = ("nc", bool(_CACHE.get("dbg")))
    if key not in _CACHE:
        _CACHE[key] = _build(bool(_CACHE.get("dbg")))
    nc = _CACHE[key]

    f32 = np.float32
    input = np.asarray(input, f32)
    hiddens = np.asarray(hiddens, f32)
    cells = np.asarray(cells, f32)
    external = np.asarray(external, f32)

    bbv = np.zeros((128, BB_COLS), NPBF)
    bbv[:, B_HT:B_XT] = _ktiles(np.ascontiguousarray(hiddens.T), N)
    bbv[:, B_XT:B_ET] = _ktiles(np.tile(input[:, None], (1, N)), N)
    bbv[:, B_ET:B_X1] = _ktiles(np.tile(external[:, None], (1, N)), N)
    bbv[:, B_X1:B_X1 + T] = _tmaj(input)

    pbv = np.zeros((128, PB_COLS), f32)
    for k, v in enumerate((g_merge, b_merge, g_i, b_i, g_o, b_o,
                           g_u, b_u, g_c, b_c)):
        pbv[:, P_GB + k * T:P_GB + (k + 1) * T] = _tmaj(v)
    pbv[0:N, P_ID:P_ID + N] = np.eye(N, dtype=f32)

    Wf_cat = np.concatenate([W_fh, W_fi], axis=1)              # [H, 4096]
    in_maps = []
    for q in range(NC):
        r = slice(q * S, (q + 1) * S)
        iou_rows = np.concatenate(
            [W_iou[g * H + q * S:g * H + (q + 1) * S, :] for g in range(3)],
            axis=0)                                            # [768, 4096]
        pv = pbv.copy()
        pv[0:N, P_CELL:P_CELL + S] = cells[:, r]
        pv[0:N, P_GF:P_GF + S] = np.tile(g_f[r], (N, 1))
        pv[0:N, P_BF:P_BF + S] = np.tile(b_f[r], (N, 1))
        pv[0:N, P_WAT:P_WAT + S] = np.tile(W_attn[0, r], (N, 1))
        pv[0:N, P_WS:P_WS + 1] = W_attn[0, r].sum()
        bv = bbv.copy()
        bv[:, B_HTC:B_HTC + 2 * N] = (
            hiddens.T[q * S:(q + 1) * S].reshape(2, 128, N)
            .transpose(1, 0, 2).reshape(128, 2 * N))
        wv = np.empty((128, WB_COLS), NPBF)
        wv[:, W_AI:W_AI + 8192] = _ktiles(
            np.ascontiguousarray(W_ai[r].T), S)
        wv[:, W_F:W_F + 8192] = _ktiles(
            np.ascontiguousarray(Wf_cat[r].T), S)
        wv[:, W_MG:W_MG + 4096] = _ktiles(
            np.ascontiguousarray(W_merge[:, r].T), H)
        wv[:, W_IOM:W_IOM + 12288] = _ktiles(
            np.ascontiguousarray(iou_rows[:, H:].T), 3 * S)
        wv[:, W_IOX:W_IOX + 12288] = _ktiles(
            np.ascontiguousarray(iou_rows[:, :H].T), 3 * S)
        in_maps.append({"wb": wv, "bb": bv, "pb": pv})

    res = run_bass_kernel_spmd(nc, in_maps, core_ids=list(range(NC)))
    _CACHE["last_results"] = res
    r0 = res.results[0]
    new_h = np.ascontiguousarray(r0["out_h"].T).reshape(H)
    new_c = np.ascontiguousarray(r0["out_c"].T).reshape(H)
    return new_h, new_c


# revision 19
# speedup vs baseline: 1.3555x; 1.0152x over previous
"""AttentiveChildSumTreeLSTMCell on 8 Trainium2 NeuronCores.

Structure (one NEFF, SPMD on 8 cores):
  * X1 cross-core allreduce via XOR-butterfly peer-to-peer SWDGE remote DMA
    (3 rounds, partner tpb^2^r): merge-projection partials M, attention
    logit partials, forget-gate LN stat partials -- one [128,528] bf16
    payload.  Bypasses ncfw (cold-start ~60us) entirely.
  * X3 final AllGather of the per-core iou/fc chunk via a warm ncfw
    collective (a tiny co-launch AllGather fires first; NEFFs without any
    collective get launched ~1.4ms staggered).
  * All full-H LayerNorms are single Q7 gpsimd.layernorm instructions; the
    `proxy` library holds layernorm + remote-dma so there is no mid-kernel
    IRAM reload.  tanh(x) = 2*sigmoid(2x)-1 keeps the scalar engine on the
    Sigmoid table; Exp/Sqrt loads are sequenced off the critical path.
  * Inputs are packed into 3 blobs per core (weights bf16 / activations
    bf16 / params f32) to minimize per-device dispatch overhead.

Matmul operands are bf16; accumulation and gate math fp32.
"""

import sys

for _p in ("/opt/trn_rl_repo",):
    if _p not in sys.path:
        sys.path.insert(0, _p)

import ml_dtypes
import numpy as np

import concourse.bacc as bacc
import concourse.bass as bass
import concourse.mybir as mybir
import concourse.tile as tile
from concourse.bass_utils import run_bass_kernel_spmd
from concourse.library_config import proxy as _proxy_lib
from concourse.tile_rust import add_dep_helper

F32 = mybir.dt.float32
BF16 = mybir.dt.bfloat16
AF = mybir.ActivationFunctionType
ALU = mybir.AluOpType
NPBF = ml_dtypes.bfloat16
AXX = mybir.AxisListType.X

H = 2048
N = 32
NC = 8
S = H // NC           # 256 per-core chunk of every sharded dim
T = H // 128          # 16 tiles of 128 along a 2048 dim
KT = 32               # k-tiles along the 4096 contraction dims
EPS = 1e-5
INV_H = 1.0 / H

C1W = 528             # X1 payload cols (512 M + 3 stats + pad)

# weight blob column offsets (bf16 cols)
W_AI, W_F, W_MG, W_IOM, W_IOX = 0, 8192, 16384, 20480, 32768
WB_COLS = 45056
# activation blob (bf16 cols): hT | xT32 | eT32 | x1 | hTc
B_HT, B_XT, B_ET, B_X1, B_HTC = 0, 512, 1024, 1536, 1552
BB_COLS = 1616
# param blob (f32 cols)
P_GB, P_ID, P_CELL, P_GF, P_BF, P_WAT, P_WS = 0, 160, 192, 448, 704, 960, 1216
PB_COLS = 1217

_CACHE = {}


def _build(dbg=False):
    nc = bacc.Bacc(None, target_bir_lowering=False, debug=False,
                   num_devices=NC)

    wb = nc.dram_tensor("wb", [128, WB_COLS], BF16, kind="ExternalInput")
    bb = nc.dram_tensor("bb", [128, BB_COLS], BF16, kind="ExternalInput")
    pb = nc.dram_tensor("pb", [128, PB_COLS], F32, kind="ExternalInput")
    out_h = nc.dram_tensor("out_h", [128, T], F32, kind="ExternalOutput")
    out_c = nc.dram_tensor("out_c", [128, T], F32, kind="ExternalOutput")
    dbg_t = {}
    if dbg:
        for nm, shp in [("d_ml", [128, T]), ("d_mhln", [128, T]),
                        ("d_iou", [1, 768]), ("d_iv", [128, T]),
                        ("d_uv", [128, T]), ("d_fcv", [128, T]),
                        ("d_iln", [128, T]), ("d_cl", [128, T]),
                        ("d_fg", [N, S]), ("d_exps", [N, 1]),
                        ("d_c1", [128, C1W]), ("d_Mfull", [128, 512]),
                        ("d_lgf", [N, 1]), ("d_fsu", [N, 1]),
                        ("d_fss", [N, 1]), ("d_s3", [128, 64])]:
            dbg_t[nm] = nc.dram_tensor(nm, shp, F32, kind="ExternalOutput")

    rsem1 = [nc.alloc_semaphore(f"x1_remote_{r}") for r in range(3)]
    lsem = nc.alloc_semaphore("x_local")
    x1_consumers = []

    with tile.TileContext(nc) as tc:
        with (
            tc.tile_pool(name="sb", bufs=1) as sb,
            tc.tile_pool(name="ps", bufs=1, space="PSUM") as ps,
            tc.tile_pool(name="dram", bufs=1, space="DRAM") as dram,
        ):
            # ---- co-launch / ncfw warm-up collective (output unused) ----
            warm_in = dram.tile([1, 64], F32, name="warm_in")
            warm_out = dram.tile([8, 64], F32, name="warm_out")
            nc.gpsimd.collective_compute(
                "AllGather", ALU.bypass,
                replica_groups=[list(range(NC))],
                ins=[warm_in.opt()], outs=[warm_out.opt()])

            # ---- input loads: activations, weights (ordered), params ----
            bb_sb = sb.tile([128, BB_COLS], BF16, name="bb_sb")
            nc.sync.dma_start(bb_sb[:], bb[:])
            wb_sb = sb.tile([128, WB_COLS], BF16, name="wb_sb")
            wdmas = []
            # arrival order: wai, wf, wmg, wioux, wioum -- ~0.5MB chunks so
            # remote-DMA descriptors interleave into the queues quickly
            ranges = ([(W_AI + k * 2048, W_AI + (k + 1) * 2048)
                       for k in range(4)]
                      + [(W_F + k * 2048, W_F + (k + 1) * 2048)
                         for k in range(4)]
                      + [(W_MG + k * 2048, W_MG + (k + 1) * 2048)
                         for k in range(2)]
                      + [(W_IOX + k * 2048, W_IOX + (k + 1) * 2048)
                         for k in range(6)]
                      + [(W_IOM + k * 2048, W_IOM + (k + 1) * 2048)
                         for k in range(6)])
            for a, b in ranges:
                wdmas.append(nc.sync.dma_start(wb_sb[:, a:b], wb[:, a:b]))
            for i in range(2, len(wdmas)):
                add_dep_helper(wdmas[i].ins, wdmas[i - 2].ins, sync=True,
                               reason="weight DMA arrival order")
            pb_sb = sb.tile([128, PB_COLS], F32, name="pb_sb")
            nc.sync.dma_start(pb_sb[:], pb[:])

            # views into the blobs
            hT_sb = bb_sb[:, B_HT:B_XT].rearrange("p (t n) -> p t n", t=T)
            xT32_sb = bb_sb[:, B_XT:B_ET].rearrange("p (t n) -> p t n", t=T)
            eT32_sb = bb_sb[:, B_ET:B_X1].rearrange("p (t n) -> p t n", t=T)
            x1_sb = bb_sb[:, B_X1:B_X1 + T]
            hTc_sb = bb_sb[:, B_HTC:B_HTC + 2 * N].rearrange(
                "p (s n) -> p s n", s=2)
            (gm_sb, bm_sb, gi_sb, bi_sb, go_sb, bo_sb, gu_sb, bu_sb,
             gc_sb, bc_sb) = (pb_sb[:, P_GB + k * T:P_GB + (k + 1) * T]
                              for k in range(10))
            id32_sb = pb_sb[0:N, P_ID:P_ID + N]
            cells_sb = pb_sb[0:N, P_CELL:P_CELL + S]
            gf_sb = pb_sb[0:N, P_GF:P_GF + S]
            bf_sb = pb_sb[0:N, P_BF:P_BF + S]
            wat_sb = pb_sb[0:N, P_WAT:P_WAT + S]
            wsum_sb = pb_sb[0:N, P_WS:P_WS + 1]

            ones32_sb = sb.tile([N, 1], F32, name="ones32_sb")
            nc.vector.memset(ones32_sb[:], 1.0)
            onesr_sb = sb.tile([1, 128], F32, name="onesr_sb")
            nc.vector.memset(onesr_sb[:], 1.0)

            # ---- Q7 proxy library (layernorm + remote-dma, no reloads) ----
            nc.gpsimd.load_library(_proxy_lib)
            lnw_in = sb.tile([128, 1], F32, name="lnw_in")
            lnw_out = sb.tile([128, 1], F32, name="lnw_out")
            nc.vector.memset(lnw_in[:], 1.0)
            nc.gpsimd.layernorm(lnw_out[:], lnw_in[:], eps=EPS,
                                subtract_mean=True)
            tl = sb.tile([1, 1], F32, name="tl")
            nc.vector.memset(tl[:], 0.5)
            sig_pre = nc.scalar.activation(tl[:], tl[:], AF.Sigmoid)

            # ---- X1 XOR-butterfly buffers + round preps/triggers ----
            c1 = sb.tile([128, C1W], BF16, name="c1")
            b1 = [sb.tile([128, C1W], BF16, name=f"b1_{r}") for r in range(3)]
            a1 = [sb.tile([128, C1W], BF16, name=f"a1_{r}") for r in range(3)]
            g3 = sb.tile([128, NC, 8], F32, name="g3")
            c3 = g3[:, 0, :]  # own chunk lands in slot 0

            t1 = []
            prev = None
            for r in range(3):
                rd = [None] * NC
                rd[1 << r] = (0, 1 << r)
                srcap = c1[:] if r == 0 else a1[r - 1][:]
                p = nc.gpsimd.remote_dma_broadcast(
                    b1[r][:], srcap, rsem1[r], lsem, rdests=rd, queue_num=0)
                if prev is not None:
                    add_dep_helper(p.ins, prev.ins, sync=True,
                                   reason="ring FIFO order")
                t = nc.gpsimd.trigger_dma(count=None, queue_num=0)
                add_dep_helper(t.ins, p.ins, sync=True,
                               reason="trigger after its prep")
                t1.append(t)
                prev = t

            # ---- attention: ai = tanh(W_ai @ [h;e]) via 2*sig(2x)-1 ----
            ps_ai = ps.tile([N, S], F32, name="ps_ai", tag="pA")
            for kt in range(KT):
                act = hT_sb if kt < T else eT32_sb
                nc.tensor.matmul(ps_ai[:], act[:, kt % T, :],
                                 wb_sb[:, W_AI + kt * S:W_AI + (kt + 1) * S],
                                 start=(kt == 0), stop=(kt == KT - 1))
            ai_sig = sb.tile([N, S], F32, name="ai_sig")
            ai_act = nc.scalar.activation(ai_sig[:], ps_ai[:], AF.Sigmoid,
                                          scale=2.0)
            add_dep_helper(ai_act.ins, sig_pre.ins, sync=True,
                           reason="sigmoid table preload first")
            # logit = sum(tanh(ai)*wat) = 2*sum(sig*wat) - sum(wat)
            aw = sb.tile([N, S], F32, name="aw")
            lg0 = sb.tile([N, 1], F32, name="lg0")
            nc.vector.tensor_tensor(aw[:], ai_sig[:], wat_sb, op=ALU.mult)
            nc.vector.tensor_reduce(lg0[:], aw[:], AXX, ALU.add)
            lg = sb.tile([N, 1], F32, name="lg")
            nc.vector.tensor_scalar(lg[:], lg0[:], 2.0, wsum_sb,
                                    op0=ALU.mult, op1=ALU.subtract)
            c1w_lg = nc.vector.tensor_copy(c1[0:N, 512:513], lg[:])

            # Exp table preload after the attention sigmoid
            exp_pre = nc.scalar.activation(tl[:], tl[:], AF.Exp)
            add_dep_helper(exp_pre.ins, ai_act.ins, sync=True,
                           reason="exp preload after attention sigmoid")

            # ---- forget-gate preactivations + stat partials ----
            ps_f = ps.tile([N, S], F32, name="ps_f", tag="pB")
            for kt in range(KT):
                act = hT_sb if kt < T else xT32_sb
                nc.tensor.matmul(ps_f[:], act[:, kt % T, :],
                                 wb_sb[:, W_F + kt * S:W_F + (kt + 1) * S],
                                 start=(kt == 0), stop=(kt == KT - 1))
            f_lin = sb.tile([N, S], F32, name="f_lin")
            fsum = sb.tile([N, 1], F32, name="fsum")
            fsq = sb.tile([N, S], F32, name="fsq")
            fss = sb.tile([N, 1], F32, name="fss")
            nc.vector.tensor_copy(f_lin[:], ps_f[:])
            nc.vector.tensor_reduce(fsum[:], f_lin[:], AXX, ALU.add)
            nc.vector.scalar_tensor_tensor(fsq[:], f_lin[:], 1.0, f_lin[:],
                                           op0=ALU.mult, op1=ALU.mult,
                                           accum_out=fss[:])
            c1w_fs = nc.vector.tensor_copy(c1[0:N, 513:514], fsum[:])
            c1w_fq = nc.vector.tensor_copy(c1[0:N, 514:515], fss[:])

            # ---- speculative merge projections M[p,t,n] (in-chunk) ----
            ps_M = ps.tile([128, T, N], F32, name="ps_M", tag="pC")
            for t in range(T):
                for s in range(2):
                    nc.tensor.matmul(
                        ps_M[:, t, :],
                        wb_sb[:, W_MG + s * H + t * 128:
                              W_MG + s * H + (t + 1) * 128],
                        hTc_sb[:, s, :],
                        start=(s == 0), stop=(s == 1))
            c1w_m = nc.vector.tensor_copy(
                c1[:, 0:512].rearrange("p (t n) -> p t n", t=T), ps_M[:])
            for w in (c1w_lg, c1w_fs, c1w_fq, c1w_m):
                add_dep_helper(t1[0].ins, w.ins, sync=True,
                               reason="X1 round0 after payload writes")
            # allreduce rounds: acc_{r} = acc_{r-1} + recv_r
            accap = c1
            for r in range(3):
                ad = nc.vector.tensor_tensor(a1[r][:], accap[:], b1[r][:],
                                             op=ALU.add)
                add_dep_helper(ad.ins, t1[r].ins, sync=True,
                               reason="add after own round trigger")
                if r < 2:
                    add_dep_helper(t1[r + 1].ins, ad.ins, sync=True,
                                   reason="next round sends the new acc")
                x1_consumers.append((ad, rsem1[r]))
                accap = a1[r]
            afull = a1[2]

            # ---- iou x-half (accumulates into ps_iou) ----
            ps_iou = ps.tile([1, 3 * S], F32, name="ps_iou", tag="pD")
            for kt in range(T):
                for c0, c1_ in ((0, 512), (512, 768)):
                    nc.tensor.matmul(
                        ps_iou[:, c0:c1_], x1_sb[:, kt:kt + 1],
                        wb_sb[:, W_IOX + kt * 768 + c0:
                              W_IOX + kt * 768 + c1_],
                        start=(kt == 0), stop=False)

            # ================= X1 consumption (afull has the sums) ========
            exps_c = sb.tile([N, 1], F32, name="exps_c")
            exps_act = nc.scalar.activation(exps_c[:], afull[0:N, 512:513],
                                            AF.Exp)
            add_dep_helper(exps_act.ins, exp_pre.ins, sync=True,
                           reason="exp after its preload")
            ps_er = ps.tile([1, N], F32, name="ps_er", tag="pE")
            nc.tensor.matmul(ps_er[:], exps_c[:], id32_sb,
                             start=True, stop=True)
            er_sb = sb.tile([1, N], F32, name="er_sb")
            nc.vector.tensor_copy(er_sb[:], ps_er[:])
            ps_eb = ps.tile([128, N], F32, name="ps_eb", tag="pF")
            nc.tensor.matmul(ps_eb[:], onesr_sb[:], er_sb[:],
                             start=True, stop=True)

            # sigmoid table back in place while the merge reduce runs
            sig_d2 = nc.scalar.activation(tl[:], tl[:], AF.Sigmoid)
            add_dep_helper(sig_d2.ins, exps_act.ins, sync=True,
                           reason="sigmoid reload after exp")

            # ml = sum_n exps_n * Mfull[:, t, n]
            eb3 = ps_eb[:].rearrange("p (one n) -> p one n",
                                     one=1).to_broadcast((128, T, N))
            msc = sb.tile([128, T, N], F32, name="msc")
            nc.vector.tensor_tensor(
                msc[:], afull[:, 0:512].rearrange("p (t n) -> p t n", t=T),
                eb3, op=ALU.mult)
            ml = sb.tile([128, T], F32, name="ml")
            nc.vector.tensor_reduce(ml[:], msc[:], AXX, ALU.add)
            mh_ln = sb.tile([128, T], F32, name="mh_ln")
            nc.gpsimd.layernorm(mh_ln[:], ml[:], gamma_ap=gm_sb,
                                beta_ap=bm_sb, eps=EPS, subtract_mean=True)
            mh_sig = sb.tile([128, T], F32, name="mh_sig")
            mh_act = nc.scalar.activation(mh_sig[:], mh_ln[:], AF.Sigmoid,
                                          scale=2.0)
            add_dep_helper(mh_act.ins, sig_d2.ins, sync=True,
                           reason="mh sigmoid after table reload")
            mh_bf = sb.tile([128, T], BF16, name="mh_bf")
            nc.vector.tensor_scalar(mh_bf[:], mh_sig[:], 2.0, -1.0,
                                    op0=ALU.mult, op1=ALU.add)

            # ---- iou mh-half (finishes ps_iou accumulation) ----
            for kt in range(T):
                for c0, c1_ in ((0, 512), (512, 768)):
                    nc.tensor.matmul(
                        ps_iou[:, c0:c1_], mh_bf[:, kt:kt + 1],
                        wb_sb[:, W_IOM + kt * 768 + c0:
                              W_IOM + kt * 768 + c1_],
                        start=False, stop=(kt == T - 1 and c0 == 512))

            # ---- f gate -> fc = sum_n f*cells (chunk) ----
            fmean = sb.tile([N, 1], F32, name="fmean")
            fmsq = sb.tile([N, 1], F32, name="fmsq")
            fvar = sb.tile([N, 1], F32, name="fvar")
            fstd = sb.tile([N, 1], F32, name="fstd")
            frstd = sb.tile([N, 1], F32, name="frstd")
            nc.vector.tensor_scalar_mul(fmean[:], afull[0:N, 513:514], INV_H)
            nc.vector.tensor_scalar_mul(fmsq[:], afull[0:N, 514:515], INV_H)
            nc.vector.tensor_tensor(fvar[:], fmean[:], fmean[:], op=ALU.mult)
            nc.vector.tensor_sub(fvar[:], fmsq[:], fvar[:])
            nc.vector.tensor_scalar_add(fvar[:], fvar[:], EPS)
            sqrt_act = nc.scalar.activation(fstd[:], fvar[:], AF.Sqrt)
            add_dep_helper(sqrt_act.ins, mh_act.ins, sync=True,
                           reason="sqrt after mh sigmoid (table order)")
            nc.vector.reciprocal(frstd[:], fstd[:])
            ft = sb.tile([N, S], F32, name="ft")
            nc.vector.tensor_scalar(ft[:], f_lin[:], fmean[:], frstd[:],
                                    op0=ALU.subtract, op1=ALU.mult)
            nc.vector.tensor_tensor(ft[:], ft[:], gf_sb, op=ALU.mult)
            nc.vector.tensor_tensor(ft[:], ft[:], bf_sb, op=ALU.add)
            f_g = sb.tile([N, S], F32, name="f_g")
            f_act = nc.scalar.activation(f_g[:], ft[:], AF.Sigmoid)
            add_dep_helper(f_act.ins, sqrt_act.ins, sync=True,
                           reason="f sigmoid after sqrt")
            fprod = sb.tile([N, S], F32, name="fprod")
            nc.vector.tensor_tensor(fprod[:], f_g[:], cells_sb, op=ALU.mult)
            ps_fc = ps.tile([128, 2], F32, name="ps_fc", tag="pE")
            for s in range(2):
                nc.tensor.matmul(ps_fc[:, s:s + 1],
                                 fprod[:, s * 128:(s + 1) * 128],
                                 ones32_sb[:], start=True, stop=True)
            c3w_fc = nc.vector.tensor_copy(c3[:, 6:8], ps_fc[:])

            iou_sb = sb.tile([1, 3 * S], F32, name="iou_sb")
            nc.vector.tensor_copy(iou_sb[:], ps_iou[:])
            # reshape [1,768] -> [128,6] t-major via 6 PE transposes
            ps_tr = ps.tile([128, 6], F32, name="ps_tr", tag="pF")
            for k in range(6):
                nc.tensor.matmul(ps_tr[:, k:k + 1],
                                 iou_sb[0:1, k * 128:(k + 1) * 128],
                                 onesr_sb[0:1, 0:1], is_transpose=True,
                                 start=True, stop=True)
            c3w_iou = nc.vector.tensor_copy(c3[:, 0:6], ps_tr[:])
            # X3: warm ncfw AllGather of the [128,8] chunk (rank order)
            ag3_in = dram.tile([1, 1024], F32, name="ag3_in")
            ag3_out = dram.tile([NC, 1024], F32, name="ag3_out")
            nc.sync.dma_start(
                ag3_in[0, :].rearrange("(p c) -> p c", p=128), c3)
            nc.gpsimd.collective_compute(
                "AllGather", ALU.bypass,
                replica_groups=[list(range(NC))],
                ins=[ag3_in.opt()], outs=[ag3_out.opt()])
            nc.sync.dma_start(
                g3[:], ag3_out[:, :].rearrange("s (p c) -> p s c", p=128))

            # ================= X3 consumption: final gates =================
            vec = {}
            for idx, nm in ((0, "iv"), (1, "ov"), (2, "uv"), (3, "fcv")):
                vt = sb.tile([128, T], F32, name=nm)
                nc.vector.tensor_copy(
                    vt[:].rearrange("p (s d) -> p s d", s=NC),
                    g3[:, :, 2 * idx:2 * idx + 2])
                vec[nm] = vt

            i_ln = sb.tile([128, T], F32, name="i_ln")
            o_ln = sb.tile([128, T], F32, name="o_ln")
            u_ln = sb.tile([128, T], F32, name="u_ln")
            nc.gpsimd.layernorm(i_ln[:], vec["iv"][:], gamma_ap=gi_sb,
                                beta_ap=bi_sb, eps=EPS, subtract_mean=True)
            nc.gpsimd.layernorm(o_ln[:], vec["ov"][:], gamma_ap=go_sb,
                                beta_ap=bo_sb, eps=EPS, subtract_mean=True)
            nc.gpsimd.layernorm(u_ln[:], vec["uv"][:], gamma_ap=gu_sb,
                                beta_ap=bu_sb, eps=EPS, subtract_mean=True)
            i_g = sb.tile([128, T], F32, name="i_g")
            o_g = sb.tile([128, T], F32, name="o_g")
            u_s = sb.tile([128, T], F32, name="u_s")
            nc.scalar.activation(i_g[:], i_ln[:], AF.Sigmoid)
            nc.scalar.activation(o_g[:], o_ln[:], AF.Sigmoid)
            nc.scalar.activation(u_s[:], u_ln[:], AF.Sigmoid, scale=2.0)
            u_g = sb.tile([128, T], F32, name="u_g")
            nc.vector.tensor_scalar(u_g[:], u_s[:], 2.0, -1.0,
                                    op0=ALU.mult, op1=ALU.add)
            cl = sb.tile([128, T], F32, name="cl")
            nc.vector.tensor_tensor(cl[:], i_g[:], u_g[:], op=ALU.mult)
            nc.vector.tensor_tensor(cl[:], cl[:], vec["fcv"][:], op=ALU.add)
            new_c = sb.tile([128, T], F32, name="new_c")
            nc.gpsimd.layernorm(new_c[:], cl[:], gamma_ap=gc_sb,
                                beta_ap=bc_sb, eps=EPS, subtract_mean=True)
            th_s = sb.tile([128, T], F32, name="th_s")
            nc.scalar.activation(th_s[:], new_c[:], AF.Sigmoid, scale=2.0)
            th = sb.tile([128, T], F32, name="th")
            nc.vector.tensor_scalar(th[:], th_s[:], 2.0, -1.0,
                                    op0=ALU.mult, op1=ALU.add)
            new_h = sb.tile([128, T], F32, name="new_h")
            nc.vector.tensor_tensor(new_h[:], o_g[:], th[:], op=ALU.mult)

            nc.sync.dma_start(out_c[:], new_c[:])
            nc.sync.dma_start(out_h[:], new_h[:])
            if dbg:
                for nm, src_t in [("d_ml", ml[:]), ("d_mhln", mh_ln[:]),
                                  ("d_iou", iou_sb[:]),
                                  ("d_iv", vec["iv"][:]),
                                  ("d_uv", vec["uv"][:]),
                                  ("d_fcv", vec["fcv"][:]),
                                  ("d_iln", i_ln[:]), ("d_cl", cl[:]),
                                  ("d_fg", f_g[:]), ("d_exps", exps_c[:]),
                                  ("d_c1", c1[:]),
                                  ("d_Mfull", afull[:, 0:512]),
                                  ("d_lgf", afull[0:N, 512:513]),
                                  ("d_fsu", afull[0:N, 513:514]),
                                  ("d_fss", afull[0:N, 514:515]),
                                  ("d_s3", g3[:].rearrange(
                                      "p s f -> p (s f)"))]:
                    dd = sb.tile(list(dbg_t[nm].shape), F32, name=nm + "_d")
                    nc.vector.tensor_copy(dd[:], src_t)
                    nc.sync.dma_start(dbg_t[nm][:], dd[:])

    # Remote-arrival waits, invisible to the single-core scheduling sim:
    # patched after tile scheduling, split into event semaphores at compile.
    for ad, sem in x1_consumers:
        bass.BassInstruction(ad.ins).wait_op(sem, 2, "sem-ge", check=False)

    nc.compile()
    return nc


def _tmaj(v):
    """[2048] vector -> [128,16] t-major sbuf image (sb[p,t] = v[t*128+p])."""
    return np.ascontiguousarray(v.reshape(T, 128).T)


def _ktiles(wT, cols):
    """wT: [K_in, out_cols] -> [128, (K_in/128)*cols] partition-major pack."""
    k_in = wT.shape[0]
    return np.ascontiguousarray(
        wT.reshape(k_in // 128, 128, cols).transpose(1, 0, 2).reshape(
            128, (k_in // 128) * cols))


def kernel(input, hiddens, cells, external,
           W_ai, W_attn, W_merge, W_iou, W_fi, W_fh,
           g_merge, b_merge, g_f, b_f, g_i, b_i, g_o, b_o, g_u, b_u,
           g_c, b_c):
    key = ("nc", bool(_CACHE.get("dbg")))
    if key not in _CACHE:
        _CACHE[key] = _build(bool(_CACHE.get("dbg")))
    nc = _CACHE[key]

    f32 = np.float32
    input = np.asarray(input, f32)
    hiddens = np.asarray(hiddens, f32)
    cells = np.asarray(cells, f32)
    external = np.asarray(external, f32)

    bbv = np.zeros((128, BB_COLS), NPBF)
    bbv[:, B_HT:B_XT] = _ktiles(np.ascontiguousarray(hiddens.T), N)
    bbv[:, B_XT:B_ET] = _ktiles(np.tile(input[:, None], (1, N)), N)
    bbv[:, B_ET:B_X1] = _ktiles(np.tile(external[:, None], (1, N)), N)
    bbv[:, B_X1:B_X1 + T] = _tmaj(input)

    pbv = np.zeros((128, PB_COLS), f32)
    for k, v in enumerate((g_merge, b_merge, g_i, b_i, g_o, b_o,
                           g_u, b_u, g_c, b_c)):
        pbv[:, P_GB + k * T:P_GB + (k + 1) * T] = _tmaj(v)
    pbv[0:N, P_ID:P_ID + N] = np.eye(N, dtype=f32)

    Wf_cat = np.concatenate([W_fh, W_fi], axis=1)              # [H, 4096]
    in_maps = []
    for q in range(NC):
        r = slice(q * S, (q + 1) * S)
        iou_rows = np.concatenate(
            [W_iou[g * H + q * S:g * H + (q + 1) * S, :] for g in range(3)],
            axis=0)                                            # [768, 4096]
        pv = pbv.copy()
        pv[0:N, P_CELL:P_CELL + S] = cells[:, r]
        pv[0:N, P_GF:P_GF + S] = np.tile(g_f[r], (N, 1))
        pv[0:N, P_BF:P_BF + S] = np.tile(b_f[r], (N, 1))
        pv[0:N, P_WAT:P_WAT + S] = np.tile(W_attn[0, r], (N, 1))
        pv[0:N, P_WS:P_WS + 1] = W_attn[0, r].sum()
        bv = bbv.copy()
        bv[:, B_HTC:B_HTC + 2 * N] = (
            hiddens.T[q * S:(q + 1) * S].reshape(2, 128, N)
            .transpose(1, 0, 2).reshape(128, 2 * N))
        wv = np.empty((128, WB_COLS), NPBF)
        wv[:, W_AI:W_AI + 8192] = _ktiles(
            np.ascontiguousarray(W_ai[r].T), S)
        wv[:, W_F:W_F + 8192] = _ktiles(
            np.ascontiguousarray(Wf_cat[r].T), S)
        wv[:, W_MG:W_MG + 4096] = _ktiles(
            np.ascontiguousarray(W_merge[:, r].T), H)
        wv[:, W_IOM:W_IOM + 12288] = _ktiles(
            np.ascontiguousarray(iou_rows[:, H:].T), 3 * S)
        wv[:, W_IOX:W_IOX + 12288] = _ktiles(
            np.ascontiguousarray(iou_rows[:, :H].T), 3 * S)
        in_maps.append({"wb": wv, "bb": bv, "pb": pv})

    res = run_bass_kernel_spmd(nc, in_maps, core_ids=list(range(NC)))
    _CACHE["last_results"] = res
    r0 = res.results[0]
    new_h = np.ascontiguousarray(r0["out_h"].T).reshape(H)
    new_c = np.ascontiguousarray(r0["out_c"].T).reshape(H)
    return new_h, new_c


# revision 20
# speedup vs baseline: 1.4495x; 1.0693x over previous
"""AttentiveChildSumTreeLSTMCell on 8 Trainium2 NeuronCores.

Structure (one NEFF, SPMD on 8 cores):
  * X1 cross-core allreduce via XOR-butterfly peer-to-peer SWDGE remote DMA
    (3 rounds, partner tpb^2^r): merge-projection partials M, attention
    logit partials, forget-gate LN stat partials -- one [128,528] bf16
    payload.  Bypasses ncfw (cold-start ~60us) entirely.
  * X3 final AllGather of the per-core iou/fc chunk via a warm ncfw
    collective (a tiny co-launch AllGather fires first; NEFFs without any
    collective get launched ~1.4ms staggered).
  * All full-H LayerNorms are single Q7 gpsimd.layernorm instructions; the
    `proxy` library holds layernorm + remote-dma so there is no mid-kernel
    IRAM reload.  tanh(x) = 2*sigmoid(2x)-1 keeps the scalar engine on the
    Sigmoid table; Exp/Sqrt loads are sequenced off the critical path.
  * Inputs are packed into 3 blobs per core (weights bf16 / activations
    bf16 / params f32) to minimize per-device dispatch overhead.

Matmul operands are bf16; accumulation and gate math fp32.
"""

import sys

for _p in ("/opt/trn_rl_repo",):
    if _p not in sys.path:
        sys.path.insert(0, _p)

import ml_dtypes
import numpy as np

import concourse.bacc as bacc
import concourse.bass as bass
import concourse.mybir as mybir
import concourse.tile as tile
from concourse.bass_utils import run_bass_kernel_spmd
from concourse.library_config import proxy as _proxy_lib
from concourse.tile_rust import add_dep_helper

F32 = mybir.dt.float32
BF16 = mybir.dt.bfloat16
AF = mybir.ActivationFunctionType
ALU = mybir.AluOpType
NPBF = ml_dtypes.bfloat16
AXX = mybir.AxisListType.X

H = 2048
N = 32
NC = 8
S = H // NC           # 256 per-core chunk of every sharded dim
T = H // 128          # 16 tiles of 128 along a 2048 dim
KT = 32               # k-tiles along the 4096 contraction dims
EPS = 1e-5
INV_H = 1.0 / H

C1W = 528             # X1 payload cols (512 M + 3 stats + pad)

# weight blob column offsets (bf16 cols)
W_AI, W_F, W_MG, W_IOM, W_IOX = 0, 8192, 16384, 20480, 32768
WB_COLS = 45056
# activation blob (bf16 cols): hT | xT32 | eT32 | x1 | hTc
B_HT, B_XT, B_ET, B_X1, B_HTC = 0, 512, 1024, 1536, 1552
BB_COLS = 1616
# param blob (f32 cols)
P_GB, P_ID, P_CELL, P_GF, P_BF, P_WAT, P_WS = 0, 160, 192, 448, 704, 960, 1216
PB_COLS = 1217

_CACHE = {}


def _build(dbg=False):
    nc = bacc.Bacc(None, target_bir_lowering=False, debug=False,
                   num_devices=NC)

    wb = nc.dram_tensor("wb", [128, WB_COLS], BF16, kind="ExternalInput")
    bb = nc.dram_tensor("bb", [128, BB_COLS], BF16, kind="ExternalInput")
    pb = nc.dram_tensor("pb", [128, PB_COLS], F32, kind="ExternalInput")
    out_h = nc.dram_tensor("out_h", [128, T], F32, kind="ExternalOutput")
    out_c = nc.dram_tensor("out_c", [128, T], F32, kind="ExternalOutput")
    dbg_t = {}
    if dbg:
        for nm, shp in [("d_ml", [128, T]), ("d_mhln", [128, T]),
                        ("d_iou", [1, 768]), ("d_iv", [128, T]),
                        ("d_uv", [128, T]), ("d_fcv", [128, T]),
                        ("d_iln", [128, T]), ("d_cl", [128, T]),
                        ("d_fg", [N, S]), ("d_exps", [N, 1]),
                        ("d_c1", [128, C1W]), ("d_Mfull", [128, 512]),
                        ("d_lgf", [N, 1]), ("d_fsu", [N, 1]),
                        ("d_fss", [N, 1]), ("d_s3", [128, 64])]:
            dbg_t[nm] = nc.dram_tensor(nm, shp, F32, kind="ExternalOutput")

    rsem1 = [nc.alloc_semaphore(f"x1_remote_{r}") for r in range(3)]
    lsem = nc.alloc_semaphore("x_local")
    x1_consumers = []

    with tile.TileContext(nc) as tc:
        with (
            tc.tile_pool(name="sb", bufs=1) as sb,
            tc.tile_pool(name="ps", bufs=1, space="PSUM") as ps,
            tc.tile_pool(name="dram", bufs=1, space="DRAM") as dram,
        ):
            # ---- co-launch / ncfw warm-up collective (output unused) ----
            warm_in = dram.tile([1, 64], F32, name="warm_in")
            warm_out = dram.tile([8, 64], F32, name="warm_out")
            nc.gpsimd.collective_compute(
                "AllGather", ALU.bypass,
                replica_groups=[list(range(NC))],
                ins=[warm_in.opt()], outs=[warm_out.opt()])

            # ---- input loads: activations, weights (ordered), params ----
            bb_sb = sb.tile([128, BB_COLS], BF16, name="bb_sb")
            nc.sync.dma_start(bb_sb[:], bb[:])
            wb_sb = sb.tile([128, WB_COLS], BF16, name="wb_sb")
            wdmas = []
            # arrival order: wai, wf, wmg, wioux, wioum (~1MB chunks)
            ranges = ([(W_AI + k * 4096, W_AI + (k + 1) * 4096)
                       for k in range(2)]
                      + [(W_F + k * 4096, W_F + (k + 1) * 4096)
                         for k in range(2)]
                      + [(W_MG + k * 2048, W_MG + (k + 1) * 2048)
                         for k in range(2)]
                      + [(W_IOX + k * 4096, W_IOX + (k + 1) * 4096)
                         for k in range(3)]
                      + [(W_IOM + k * 4096, W_IOM + (k + 1) * 4096)
                         for k in range(3)])
            for a, b in ranges:
                wdmas.append(nc.sync.dma_start(wb_sb[:, a:b], wb[:, a:b]))
            for i in range(2, len(wdmas)):
                add_dep_helper(wdmas[i].ins, wdmas[i - 2].ins, sync=True,
                               reason="weight DMA arrival order")
            pb_sb = sb.tile([128, PB_COLS], F32, name="pb_sb")
            nc.sync.dma_start(pb_sb[:], pb[:])

            # views into the blobs
            hT_sb = bb_sb[:, B_HT:B_XT].rearrange("p (t n) -> p t n", t=T)
            xT32_sb = bb_sb[:, B_XT:B_ET].rearrange("p (t n) -> p t n", t=T)
            eT32_sb = bb_sb[:, B_ET:B_X1].rearrange("p (t n) -> p t n", t=T)
            x1_sb = bb_sb[:, B_X1:B_X1 + T]
            hTc_sb = bb_sb[:, B_HTC:B_HTC + 2 * N].rearrange(
                "p (s n) -> p s n", s=2)
            (gm_sb, bm_sb, gi_sb, bi_sb, go_sb, bo_sb, gu_sb, bu_sb,
             gc_sb, bc_sb) = (pb_sb[:, P_GB + k * T:P_GB + (k + 1) * T]
                              for k in range(10))
            id32_sb = pb_sb[0:N, P_ID:P_ID + N]
            cells_sb = pb_sb[0:N, P_CELL:P_CELL + S]
            gf_sb = pb_sb[0:N, P_GF:P_GF + S]
            bf_sb = pb_sb[0:N, P_BF:P_BF + S]
            wat_sb = pb_sb[0:N, P_WAT:P_WAT + S]
            wsum_sb = pb_sb[0:N, P_WS:P_WS + 1]

            ones32_sb = sb.tile([N, 1], F32, name="ones32_sb")
            nc.vector.memset(ones32_sb[:], 1.0)
            onesr_sb = sb.tile([1, 128], F32, name="onesr_sb")
            nc.vector.memset(onesr_sb[:], 1.0)

            # ---- Q7 proxy library (layernorm + remote-dma, no reloads) ----
            nc.gpsimd.load_library(_proxy_lib)
            lnw_in = sb.tile([128, 1], F32, name="lnw_in")
            lnw_out = sb.tile([128, 1], F32, name="lnw_out")
            nc.vector.memset(lnw_in[:], 1.0)
            nc.gpsimd.layernorm(lnw_out[:], lnw_in[:], eps=EPS,
                                subtract_mean=True)
            tl = sb.tile([1, 1], F32, name="tl")
            nc.vector.memset(tl[:], 0.5)
            sig_pre = nc.scalar.activation(tl[:], tl[:], AF.Sigmoid)

            # ---- X1 XOR-butterfly buffers + round preps/triggers ----
            c1 = sb.tile([128, C1W], BF16, name="c1")
            b1 = [sb.tile([128, C1W], BF16, name=f"b1_{r}") for r in range(3)]
            a1 = [sb.tile([128, C1W], BF16, name=f"a1_{r}") for r in range(3)]
            g3 = sb.tile([128, NC, 8], F32, name="g3")
            c3 = g3[:, 0, :]  # own chunk lands in slot 0

            t1 = []
            prev = None
            for r in range(3):
                rd = [None] * NC
                rd[1 << r] = (0, 1 << r)
                srcap = c1[:] if r == 0 else a1[r - 1][:]
                p = nc.gpsimd.remote_dma_broadcast(
                    b1[r][:], srcap, rsem1[r], lsem, rdests=rd, queue_num=0)
                if prev is not None:
                    add_dep_helper(p.ins, prev.ins, sync=True,
                                   reason="ring FIFO order")
                t = nc.gpsimd.trigger_dma(count=None, queue_num=0)
                add_dep_helper(t.ins, p.ins, sync=True,
                               reason="trigger after its prep")
                t1.append(t)
                prev = t

            # ---- attention: ai = tanh(W_ai @ [h;e]) via 2*sig(2x)-1 ----
            ps_ai = ps.tile([N, S], F32, name="ps_ai", tag="pA")
            for kt in range(KT):
                act = hT_sb if kt < T else eT32_sb
                nc.tensor.matmul(ps_ai[:], act[:, kt % T, :],
                                 wb_sb[:, W_AI + kt * S:W_AI + (kt + 1) * S],
                                 start=(kt == 0), stop=(kt == KT - 1))
            ai_sig = sb.tile([N, S], F32, name="ai_sig")
            ai_act = nc.scalar.activation(ai_sig[:], ps_ai[:], AF.Sigmoid,
                                          scale=2.0)
            add_dep_helper(ai_act.ins, sig_pre.ins, sync=True,
                           reason="sigmoid table preload first")
            # logit = sum(tanh(ai)*wat) = 2*sum(sig*wat) - sum(wat)
            aw = sb.tile([N, S], F32, name="aw")
            lg0 = sb.tile([N, 1], F32, name="lg0")
            nc.vector.tensor_tensor(aw[:], ai_sig[:], wat_sb, op=ALU.mult)
            nc.vector.tensor_reduce(lg0[:], aw[:], AXX, ALU.add)
            lg = sb.tile([N, 1], F32, name="lg")
            nc.vector.tensor_scalar(lg[:], lg0[:], 2.0, wsum_sb,
                                    op0=ALU.mult, op1=ALU.subtract)
            c1w_lg = nc.vector.tensor_copy(c1[0:N, 512:513], lg[:])

            # Exp table preload after the attention sigmoid
            exp_pre = nc.scalar.activation(tl[:], tl[:], AF.Exp)
            add_dep_helper(exp_pre.ins, ai_act.ins, sync=True,
                           reason="exp preload after attention sigmoid")

            # ---- forget-gate preactivations + stat partials ----
            ps_f = ps.tile([N, S], F32, name="ps_f", tag="pB")
            for kt in range(KT):
                act = hT_sb if kt < T else xT32_sb
                nc.tensor.matmul(ps_f[:], act[:, kt % T, :],
                                 wb_sb[:, W_F + kt * S:W_F + (kt + 1) * S],
                                 start=(kt == 0), stop=(kt == KT - 1))
            f_lin = sb.tile([N, S], F32, name="f_lin")
            fsum = sb.tile([N, 1], F32, name="fsum")
            fsq = sb.tile([N, S], F32, name="fsq")
            fss = sb.tile([N, 1], F32, name="fss")
            nc.vector.tensor_copy(f_lin[:], ps_f[:])
            nc.vector.tensor_reduce(fsum[:], f_lin[:], AXX, ALU.add)
            nc.vector.scalar_tensor_tensor(fsq[:], f_lin[:], 1.0, f_lin[:],
                                           op0=ALU.mult, op1=ALU.mult,
                                           accum_out=fss[:])
            c1w_fs = nc.vector.tensor_copy(c1[0:N, 513:514], fsum[:])
            c1w_fq = nc.vector.tensor_copy(c1[0:N, 514:515], fss[:])

            # ---- speculative merge projections M[p,t,n] (in-chunk) ----
            ps_M = ps.tile([128, T, N], F32, name="ps_M", tag="pC")
            for t in range(T):
                for s in range(2):
                    nc.tensor.matmul(
                        ps_M[:, t, :],
                        wb_sb[:, W_MG + s * H + t * 128:
                              W_MG + s * H + (t + 1) * 128],
                        hTc_sb[:, s, :],
                        start=(s == 0), stop=(s == 1))
            c1w_m = nc.vector.tensor_copy(
                c1[:, 0:512].rearrange("p (t n) -> p t n", t=T), ps_M[:])
            for w in (c1w_lg, c1w_fs, c1w_fq, c1w_m):
                add_dep_helper(t1[0].ins, w.ins, sync=True,
                               reason="X1 round0 after payload writes")
            # allreduce rounds: acc_{r} = acc_{r-1} + recv_r
            accap = c1
            for r in range(3):
                ad = nc.vector.tensor_tensor(a1[r][:], accap[:], b1[r][:],
                                             op=ALU.add)
                add_dep_helper(ad.ins, t1[r].ins, sync=True,
                               reason="add after own round trigger")
                if r < 2:
                    add_dep_helper(t1[r + 1].ins, ad.ins, sync=True,
                                   reason="next round sends the new acc")
                x1_consumers.append((ad, rsem1[r]))
                accap = a1[r]
            afull = a1[2]

            # ---- iou x-half (accumulates into ps_iou) ----
            ps_iou = ps.tile([1, 3 * S], F32, name="ps_iou", tag="pD")
            for kt in range(T):
                for c0, c1_ in ((0, 512), (512, 768)):
                    nc.tensor.matmul(
                        ps_iou[:, c0:c1_], x1_sb[:, kt:kt + 1],
                        wb_sb[:, W_IOX + kt * 768 + c0:
                              W_IOX + kt * 768 + c1_],
                        start=(kt == 0), stop=False)

            # ================= X1 consumption (afull has the sums) ========
            exps_c = sb.tile([N, 1], F32, name="exps_c")
            exps_act = nc.scalar.activation(exps_c[:], afull[0:N, 512:513],
                                            AF.Exp)
            add_dep_helper(exps_act.ins, exp_pre.ins, sync=True,
                           reason="exp after its preload")
            ps_er = ps.tile([1, N], F32, name="ps_er", tag="pE")
            nc.tensor.matmul(ps_er[:], exps_c[:], id32_sb,
                             start=True, stop=True)
            er_sb = sb.tile([1, N], F32, name="er_sb")
            nc.vector.tensor_copy(er_sb[:], ps_er[:])
            ps_eb = ps.tile([128, N], F32, name="ps_eb", tag="pF")
            nc.tensor.matmul(ps_eb[:], onesr_sb[:], er_sb[:],
                             start=True, stop=True)

            # sigmoid table back in place while the merge reduce runs
            sig_d2 = nc.scalar.activation(tl[:], tl[:], AF.Sigmoid)
            add_dep_helper(sig_d2.ins, exps_act.ins, sync=True,
                           reason="sigmoid reload after exp")

            # ml = sum_n exps_n * Mfull[:, t, n]
            eb3 = ps_eb[:].rearrange("p (one n) -> p one n",
                                     one=1).to_broadcast((128, T, N))
            msc = sb.tile([128, T, N], F32, name="msc")
            nc.vector.tensor_tensor(
                msc[:], afull[:, 0:512].rearrange("p (t n) -> p t n", t=T),
                eb3, op=ALU.mult)
            ml = sb.tile([128, T], F32, name="ml")
            nc.vector.tensor_reduce(ml[:], msc[:], AXX, ALU.add)
            mh_ln = sb.tile([128, T], F32, name="mh_ln")
            nc.gpsimd.layernorm(mh_ln[:], ml[:], gamma_ap=gm_sb,
                                beta_ap=bm_sb, eps=EPS, subtract_mean=True)
            mh_sig = sb.tile([128, T], F32, name="mh_sig")
            mh_act = nc.scalar.activation(mh_sig[:], mh_ln[:], AF.Sigmoid,
                                          scale=2.0)
            add_dep_helper(mh_act.ins, sig_d2.ins, sync=True,
                           reason="mh sigmoid after table reload")
            mh_bf = sb.tile([128, T], BF16, name="mh_bf")
            nc.vector.tensor_scalar(mh_bf[:], mh_sig[:], 2.0, -1.0,
                                    op0=ALU.mult, op1=ALU.add)

            # ---- iou mh-half (finishes ps_iou accumulation) ----
            for kt in range(T):
                for c0, c1_ in ((0, 512), (512, 768)):
                    nc.tensor.matmul(
                        ps_iou[:, c0:c1_], mh_bf[:, kt:kt + 1],
                        wb_sb[:, W_IOM + kt * 768 + c0:
                              W_IOM + kt * 768 + c1_],
                        start=False, stop=(kt == T - 1 and c0 == 512))

            # ---- f gate -> fc = sum_n f*cells (chunk) ----
            fmean = sb.tile([N, 1], F32, name="fmean")
            fmsq = sb.tile([N, 1], F32, name="fmsq")
            fvar = sb.tile([N, 1], F32, name="fvar")
            fstd = sb.tile([N, 1], F32, name="fstd")
            frstd = sb.tile([N, 1], F32, name="frstd")
            nc.vector.tensor_scalar_mul(fmean[:], afull[0:N, 513:514], INV_H)
            nc.vector.tensor_scalar_mul(fmsq[:], afull[0:N, 514:515], INV_H)
            nc.vector.tensor_tensor(fvar[:], fmean[:], fmean[:], op=ALU.mult)
            nc.vector.tensor_sub(fvar[:], fmsq[:], fvar[:])
            nc.vector.tensor_scalar_add(fvar[:], fvar[:], EPS)
            sqrt_act = nc.scalar.activation(fstd[:], fvar[:], AF.Sqrt)
            add_dep_helper(sqrt_act.ins, mh_act.ins, sync=True,
                           reason="sqrt after mh sigmoid (table order)")
            nc.vector.reciprocal(frstd[:], fstd[:])
            ft = sb.tile([N, S], F32, name="ft")
            nc.vector.tensor_scalar(ft[:], f_lin[:], fmean[:], frstd[:],
                                    op0=ALU.subtract, op1=ALU.mult)
            nc.vector.tensor_tensor(ft[:], ft[:], gf_sb, op=ALU.mult)
            nc.vector.tensor_tensor(ft[:], ft[:], bf_sb, op=ALU.add)
            f_g = sb.tile([N, S], F32, name="f_g")
            f_act = nc.scalar.activation(f_g[:], ft[:], AF.Sigmoid)
            add_dep_helper(f_act.ins, sqrt_act.ins, sync=True,
                           reason="f sigmoid after sqrt")
            fprod = sb.tile([N, S], F32, name="fprod")
            nc.vector.tensor_tensor(fprod[:], f_g[:], cells_sb, op=ALU.mult)
            ps_fc = ps.tile([128, 2], F32, name="ps_fc", tag="pE")
            for s in range(2):
                nc.tensor.matmul(ps_fc[:, s:s + 1],
                                 fprod[:, s * 128:(s + 1) * 128],
                                 ones32_sb[:], start=True, stop=True)
            c3w_fc = nc.vector.tensor_copy(c3[:, 6:8], ps_fc[:])

            iou_sb = sb.tile([1, 3 * S], F32, name="iou_sb")
            nc.vector.tensor_copy(iou_sb[:], ps_iou[:])
            # reshape [1,768] -> [128,6] t-major via 6 PE transposes
            ps_tr = ps.tile([128, 6], F32, name="ps_tr", tag="pF")
            for k in range(6):
                nc.tensor.matmul(ps_tr[:, k:k + 1],
                                 iou_sb[0:1, k * 128:(k + 1) * 128],
                                 onesr_sb[0:1, 0:1], is_transpose=True,
                                 start=True, stop=True)
            c3w_iou = nc.vector.tensor_copy(c3[:, 0:6], ps_tr[:])
            # X3: warm ncfw AllGather of the [128,8] chunk (rank order)
            ag3_in = dram.tile([1, 1024], F32, name="ag3_in")
            ag3_out = dram.tile([NC, 1024], F32, name="ag3_out")
            nc.sync.dma_start(
                ag3_in[0, :].rearrange("(p c) -> p c", p=128), c3)
            nc.gpsimd.collective_compute(
                "AllGather", ALU.bypass,
                replica_groups=[list(range(NC))],
                ins=[ag3_in.opt()], outs=[ag3_out.opt()])
            nc.sync.dma_start(
                g3[:], ag3_out[:, :].rearrange("s (p c) -> p s c", p=128))

            # ================= X3 consumption: final gates =================
            vec = {}
            for idx, nm in ((0, "iv"), (1, "ov"), (2, "uv"), (3, "fcv")):
                vt = sb.tile([128, T], F32, name=nm)
                nc.vector.tensor_copy(
                    vt[:].rearrange("p (s d) -> p s d", s=NC),
                    g3[:, :, 2 * idx:2 * idx + 2])
                vec[nm] = vt

            i_ln = sb.tile([128, T], F32, name="i_ln")
            o_ln = sb.tile([128, T], F32, name="o_ln")
            u_ln = sb.tile([128, T], F32, name="u_ln")
            nc.gpsimd.layernorm(i_ln[:], vec["iv"][:], gamma_ap=gi_sb,
                                beta_ap=bi_sb, eps=EPS, subtract_mean=True)
            nc.gpsimd.layernorm(o_ln[:], vec["ov"][:], gamma_ap=go_sb,
                                beta_ap=bo_sb, eps=EPS, subtract_mean=True)
            nc.gpsimd.layernorm(u_ln[:], vec["uv"][:], gamma_ap=gu_sb,
                                beta_ap=bu_sb, eps=EPS, subtract_mean=True)
            i_g = sb.tile([128, T], F32, name="i_g")
            o_g = sb.tile([128, T], F32, name="o_g")
            u_s = sb.tile([128, T], F32, name="u_s")
            nc.scalar.activation(i_g[:], i_ln[:], AF.Sigmoid)
            nc.scalar.activation(o_g[:], o_ln[:], AF.Sigmoid)
            nc.scalar.activation(u_s[:], u_ln[:], AF.Sigmoid, scale=2.0)
            u_g = sb.tile([128, T], F32, name="u_g")
            nc.vector.tensor_scalar(u_g[:], u_s[:], 2.0, -1.0,
                                    op0=ALU.mult, op1=ALU.add)
            cl = sb.tile([128, T], F32, name="cl")
            nc.vector.tensor_tensor(cl[:], i_g[:], u_g[:], op=ALU.mult)
            nc.vector.tensor_tensor(cl[:], cl[:], vec["fcv"][:], op=ALU.add)
            new_c = sb.tile([128, T], F32, name="new_c")
            nc.gpsimd.layernorm(new_c[:], cl[:], gamma_ap=gc_sb,
                                beta_ap=bc_sb, eps=EPS, subtract_mean=True)
            th_s = sb.tile([128, T], F32, name="th_s")
            nc.scalar.activation(th_s[:], new_c[:], AF.Sigmoid, scale=2.0)
            th = sb.tile([128, T], F32, name="th")
            nc.vector.tensor_scalar(th[:], th_s[:], 2.0, -1.0,
                                    op0=ALU.mult, op1=ALU.add)
            new_h = sb.tile([128, T], F32, name="new_h")
            nc.vector.tensor_tensor(new_h[:], o_g[:], th[:], op=ALU.mult)

            nc.sync.dma_start(out_c[:], new_c[:])
            nc.sync.dma_start(out_h[:], new_h[:])
            if dbg:
                for nm, src_t in [("d_ml", ml[:]), ("d_mhln", mh_ln[:]),
                                  ("d_iou", iou_sb[:]),
                                  ("d_iv", vec["iv"][:]),
                                  ("d_uv", vec["uv"][:]),
                                  ("d_fcv", vec["fcv"][:]),
                                  ("d_iln", i_ln[:]), ("d_cl", cl[:]),
                                  ("d_fg", f_g[:]), ("d_exps", exps_c[:]),
                                  ("d_c1", c1[:]),
                                  ("d_Mfull", afull[:, 0:512]),
                                  ("d_lgf", afull[0:N, 512:513]),
                                  ("d_fsu", afull[0:N, 513:514]),
                                  ("d_fss", afull[0:N, 514:515]),
                                  ("d_s3", g3[:].rearrange(
                                      "p s f -> p (s f)"))]:
                    dd = sb.tile(list(dbg_t[nm].shape), F32, name=nm + "_d")
                    nc.vector.tensor_copy(dd[:], src_t)
                    nc.sync.dma_start(dbg_t[nm][:], dd[:])

    # Remote-arrival waits, invisible to the single-core scheduling sim:
    # patched after tile scheduling, split into event semaphores at compile.
    for ad, sem in x1_consumers:
        bass.BassInstruction(ad.ins).wait_op(sem, 2, "sem-ge", check=False)

    nc.compile()
    return nc


def _tmaj(v):
    """[2048] vector -> [128,16] t-major sbuf image (sb[p,t] = v[t*128+p])."""
    return np.ascontiguousarray(v.reshape(T, 128).T)


def _ktiles(wT, cols):
    """wT: [K_in, out_cols] -> [128, (K_in/128)*cols] partition-major pack."""
    k_in = wT.shape[0]
    return np.ascontiguousarray(
        wT.reshape(k_in // 128, 128, cols).transpose(1, 0, 2).reshape(
            128, (k_in // 128) * cols))


def kernel(input, hiddens, cells, external,
           W_ai, W_attn, W_merge, W_iou, W_fi, W_fh,
           g_merge, b_merge, g_f, b_f, g_i, b_i, g_o, b_o, g_u, b_u,
           g_c, b_c):
    key = ("nc", bool(_CACHE.get("dbg")))
    if key not in _CACHE:
        _CACHE[key] = _build(bool(_CACHE.get("dbg")))
    nc = _CACHE[key]

    f32 = np.float32
    input = np.asarray(input, f32)
    hiddens = np.asarray(hiddens, f32)
    cells = np.asarray(cells, f32)
    external = np.asarray(external, f32)

    bbv = np.zeros((128, BB_COLS), NPBF)
    bbv[:, B_HT:B_XT] = _ktiles(np.ascontiguousarray(hiddens.T), N)
    bbv[:, B_XT:B_ET] = _ktiles(np.tile(input[:, None], (1, N)), N)
    bbv[:, B_ET:B_X1] = _ktiles(np.tile(external[:, None], (1, N)), N)
    bbv[:, B_X1:B_X1 + T] = _tmaj(input)

    pbv = np.zeros((128, PB_COLS), f32)
    for k, v in enumerate((g_merge, b_merge, g_i, b_i, g_o, b_o,
                           g_u, b_u, g_c, b_c)):
        pbv[:, P_GB + k * T:P_GB + (k + 1) * T] = _tmaj(v)
    pbv[0:N, P_ID:P_ID + N] = np.eye(N, dtype=f32)

    Wf_cat = np.concatenate([W_fh, W_fi], axis=1)              # [H, 4096]
    in_maps = []
    for q in range(NC):
        r = slice(q * S, (q + 1) * S)
        iou_rows = np.concatenate(
            [W_iou[g * H + q * S:g * H + (q + 1) * S, :] for g in range(3)],
            axis=0)                                            # [768, 4096]
        pv = pbv.copy()
        pv[0:N, P_CELL:P_CELL + S] = cells[:, r]
        pv[0:N, P_GF:P_GF + S] = np.tile(g_f[r], (N, 1))
        pv[0:N, P_BF:P_BF + S] = np.tile(b_f[r], (N, 1))
        pv[0:N, P_WAT:P_WAT + S] = np.tile(W_attn[0, r], (N, 1))
        pv[0:N, P_WS:P_WS + 1] = W_attn[0, r].sum()
        bv = bbv.copy()
        bv[:, B_HTC:B_HTC + 2 * N] = (
            hiddens.T[q * S:(q + 1) * S].reshape(2, 128, N)
            .transpose(1, 0, 2).reshape(128, 2 * N))
        wv = np.empty((128, WB_COLS), NPBF)
        wv[:, W_AI:W_AI + 8192] = _ktiles(
            np.ascontiguousarray(W_ai[r].T), S)
        wv[:, W_F:W_F + 8192] = _ktiles(
            np.ascontiguousarray(Wf_cat[r].T), S)
        wv[:, W_MG:W_MG + 4096] = _ktiles(
            np.ascontiguousarray(W_merge[:, r].T), H)
        wv[:, W_IOM:W_IOM + 12288] = _ktiles(
            np.ascontiguousarray(iou_rows[:, H:].T), 3 * S)
        wv[:, W_IOX:W_IOX + 12288] = _ktiles(
            np.ascontiguousarray(iou_rows[:, :H].T), 3 * S)
        in_maps.append({"wb": wv, "bb": bv, "pb": pv})

    res = run_bass_kernel_spmd(nc, in_maps, core_ids=list(range(NC)))
    _CACHE["last_results"] = res
    r0 = res.results[0]
    new_h = np.ascontiguousarray(r0["out_h"].T).reshape(H)
    new_c = np.ascontiguousarray(r0["out_c"].T).reshape(H)
    return new_h, new_c
